# revision 23
# baseline (speedup 1.0000x reference)
"""Trainium2 Bass kernel for nn_MISA (dense_transformer, data-parallel over 8 cores).

Layout: feature-major activations [feat_part=128, mtile, batch_cols] per core.
Batch 4096 -> 512 per core -> two passes of 256 columns.
All matmuls bf16 (fp32 PSUM accumulation); LN/softmax internals fp32.

v3 (PE-gap removal): the PE is the bottleneck engine; v2 lost ~400us to PE
idle gaps waiting on DVE softmax/LN chains plus ~300us of HAM cold-throttle
restarts those gaps caused.  v3 keeps the PE warm:
- every projection is a generator yielding at m-tile boundaries; a Feed
  object interleaves pending projection chunks into every DVE-bound window
  (softmax, LayerNorm, gate chains) as PE filler.
- attention loops are software-pipelined: scores(e1) / softmax(e1) /
  filler / av(e1-1), so av's dependency on softmax is always satisfied by
  the time the PE reaches it.
- av accumulation adds run on GpSimd (otherwise idle), LN's per-tile
  affine (g,b) runs on the Scalar engine, self-attn residual adds moved
  from PE identity-matmuls to DVE.
- pass c+1's head (x load, expand, kv0, q0) is pumped as PE filler during
  pass c's tail (wo1/wo2/final LN), removing the inter-pass PE gap.

Structural simplifications (exact, not approximations):
- attention with all-equal keys/values (q/k/v = broadcast joint row) is the
  identity on v: cross_tj == cross_sj == out_proj4(v_proj4(joint)).
- mean over query positions commutes with out_proj and with A@V, so the six
  cross outputs never materialize per-query outputs (abar-weighted V only).
- all-equal queries (j as q): single query row, output equals its mean.
"""
import sys, math
from contextlib import ExitStack
sys.path.insert(0, "/opt/trn_rl_repo")

import numpy as np
import ml_dtypes

import concourse.bass as bass
import concourse.mybir as mybir
from concourse import bacc
import concourse.tile as tile
from concourse import bass_utils

F32 = mybir.dt.float32
BF16 = mybir.dt.bfloat16
AF = mybir.ActivationFunctionType
ALU = mybir.AluOpType
BF = ml_dtypes.bfloat16

H = 8
E = 4
HD = 1024
B = 4096
NCORES = 8
BC = B // NCORES          # 512 batch per core
NP = 2                    # passes per core
N = BC // NP              # 256 batch cols per pass
EPS = 1e-5


def _bias_cols(b):
    # [M] -> [128, M//128]: column m = per-partition bias of m-tile m
    return np.ascontiguousarray(np.asarray(b, np.float32).reshape(-1, 128).T)


class Feed:
    """Queue of projection generators (or factories) pumped as PE filler.
    Factories instantiate lazily when reached, so their eager first weight
    DMA overlaps the tail of the preceding stream."""
    def __init__(self, *gens):
        self.q = list(gens)

    def add(self, g):
        self.q.append(g)

    def pump(self, n=1):
        while n > 0 and self.q:
            g = self.q[0]
            if not hasattr(g, "__next__"):
                g = self.q[0] = g()
            try:
                next(g)
                n -= 1
            except StopIteration:
                self.q.pop(0)

    def drain(self):
        while self.q:
            self.pump(1)


def drain(g):
    for _ in g:
        pass


def build(res_w: float):
    nc = bacc.Bacc("TRN2", target_bir_lowering=False, debug=False)

    def din(name, shape, dt):
        return nc.dram_tensor(name, list(shape), dt, kind="ExternalInput").ap()

    xt_d = din("xt", (128, 8, BC), F32)
    xs_d = din("xs", (128, 8, BC), F32)
    # weights pair-blocked: [128, M/256, K/128, 256] — one (2-mtile, all-kt)
    # block is contiguous per partition, so block DMAs run at full rate
    wexp = [din(f"wexp{j}", (128, 16, 8, 256), BF16) for j in range(2)]
    bexp = [din(f"bexp{j}", (128, 32), F32) for j in range(2)]
    wqkv = [din(f"wqkv{i}", (128, 12, 8, 256), BF16) for i in range(5)]
    bqkv = [din(f"bqkv{i}", (128, 24), F32) for i in range(5)]
    wout = [din(f"wout{i}", (128, 4, 8, 256), BF16) for i in range(5)]
    bout = [din(f"bout{i}", (128, 8), F32) for i in range(5)]
    wjoint = din("wjoint", (128, 4, 16, 256), BF16)
    bjoint = din("bjoint", (128, 8), F32)
    wgate = [din(f"wgate{g}", (128, 4, 16, 256), BF16) for g in range(3)]
    bgate = [din(f"bgate{g}", (128, 8), F32) for g in range(3)]
    wo1 = din("wo1", (128, 8, 48, 256), BF16)
    bo1 = din("bo1", (128, 16), F32)
    wo2 = din("wo2", (128, 4, 16, 256), BF16)
    bo2 = din("bo2", (128, 8), F32)
    lng = [din(f"lng{i}", (128, 8), F32) for i in range(3)]
    lnb = [din(f"lnb{i}", (128, 8), F32) for i in range(3)]
    sel_d = din("sel_c", (8, 8 * 128), BF16)
    o32_d = din("o32_c", (128, 64), BF16)
    y_d = nc.dram_tensor("y", [128, 8, BC], BF16, kind="ExternalOutput").ap()

    with tile.TileContext(nc) as tc, ExitStack() as ctx:
        P = lambda **kw: ctx.enter_context(tc.tile_pool(**kw))
        cst = P(name="cst", bufs=1)
        wgp = P(name="wgp", bufs=3)                 # [128,8,256] weight blocks
        mmp = P(name="mmp", bufs=4, space="PSUM")   # 4 x [128,512] banks
        scp = P(name="scp", bufs=1, space="PSUM")   # [8,4,N] = 2 banks
        brp = P(name="brp", bufs=2, space="PSUM")   # 2 x [128,N] banks
        expp = P(name="expp", bufs=2)   # xp_t, xp_s; also h1, next xp_t
        enhp = P(name="enhp", bufs=2)   # t_enh, s_enh [128,8,4,N]
        qkvp = P(name="qkvp", bufs=2)   # k4, v4 [128,4,8,N]
        acc4p = P(name="acc4p", bufs=2)  # qa tiles + jacc [128,4,8,N]
        bigp = P(name="bigp", bufs=7)   # all long-lived [128,8,N] bf16
        actp = meanp = accp = gatep = bigp
        qkp = P(name="qkp", bufs=2)     # qk products, av curs, ln scratch
        xbp = P(name="xbp", bufs=3)     # xt_b, xs_b, osum
        smp = P(name="smp", bufs=1)     # softmax exp [8,4,N] bf16
        smdp = P(name="smdp", bufs=2)   # softmax denom [8,N] f32
        smbp = P(name="smbp", bufs=3)   # a_l bf16 [8,N]
        smrp = P(name="smrp", bufs=3)   # LN scalars [1,N] f32
        smabp = P(name="smabp", bufs=4)  # abar f32 [8,N]

        nc._phase_marks = []
        def mark(name):
            nc._phase_marks.append((name, nc.next_id()))
        nc.mark = mark

        _tc = [0]
        def T(pool, shape, dtype, tag):
            _tc[0] += 1
            return pool.tile(shape, dtype, tag=tag, name=f"{tag}_{_tc[0]}")

        ones_b = T(cst, [128, 1], BF16, "ones_b")
        nc.any.memset(ones_b[:], 1.0)
        onerow_f = T(cst, [1, 128], F32, "onerow_f")
        nc.any.memset(onerow_f[:], 1.0)
        sel = T(cst, [8, 8 * 128], BF16, "sel")
        nc.sync.dma_start(out=sel[:], in_=sel_d)
        o32 = T(cst, [128, 64], BF16, "o32")   # all-ones column at col 32
        nc.sync.dma_start(out=o32[:], in_=o32_d)
        eps_t = T(cst, [1, 1], F32, "eps_t")
        nc.any.memset(eps_t[:], EPS)

        def ctile(name, ap):
            t = cst.tile(list(ap.shape), ap.dtype, tag=name)
            nc.sync.dma_start(out=t[:], in_=ap)
            return t

        bexp_t = [ctile(f"bexp{j}", bexp[j]) for j in range(2)]
        bqkv_t = [ctile(f"bqkv{i}", bqkv[i]) for i in range(5)]
        bout_t = [ctile(f"bout{i}", bout[i]) for i in range(5)]
        bjoint_t = ctile("bjoint", bjoint)
        bgate_t = [ctile(f"bgate{g}", bgate[g]) for g in range(3)]
        bo1_t = ctile("bo1", bo1)
        bo2_t = ctile("bo2", bo2)
        lng_t = [ctile(f"lng{i}", lng[i]) for i in range(3)]
        lnb_t = [ctile(f"lnb{i}", lnb[i]) for i in range(3)]

        def gen_projS(w_d, M, src_pair, evict2, wcol0=0, npair=2):
            """Shared-weight projection, K=1024. src_pair(kt, p) -> [128,2,N]
            moving pair; two pairs (4 sources) per stationary load.
            evict2(mj, psums): psums[p] = [128,512] = pair p's two outputs.
            Yields once per mj (16 matmuls).  The first weight DMA issues
            eagerly at call time; later blocks prefetch one group ahead."""
            nmt = M // 128
            blocks = list(range(0, nmt, 2))
            wts = {}
            def load(mj0, split=False):
                wt = T(wgp, [128, 8, 256], BF16, "wg")
                blk = w_d[:, (wcol0 + mj0 * 128) // 256, :, :]
                if split:
                    nc.sync.dma_start(out=wt[:, 0:4, :], in_=blk[:, 0:4, :])
                    nc.sync.dma_start(out=wt[:, 4:8, :], in_=blk[:, 4:8, :])
                else:
                    nc.sync.dma_start(out=wt[:], in_=blk)
                wts[mj0] = wt
            load(blocks[0], split=True)
            def g():
                for bi, mj0 in enumerate(blocks):
                    gm = min(2, nmt - mj0)
                    wt = wts.pop(mj0)
                    for mj in range(mj0, mj0 + gm):
                        ps = [T(mmp, [128, 512], F32, "mm")
                              for _ in range(npair)]
                        for kt in range(8):
                            w_sl = wt[:, kt,
                                      (mj - mj0) * 128 : (mj - mj0 + 1) * 128]
                            for p in range(npair):
                                nc.tensor.matmul(ps[p][:], w_sl,
                                                 src_pair(kt, p),
                                                 start=(kt == 0),
                                                 stop=(kt == 7))
                        evict2(mj, ps)
                        if mj == mj0 and bi + 1 < len(blocks):
                            load(blocks[bi + 1])
                        yield
            return g()

        def gen_projM(w_d, M, K, src, evict2m, wcol0=0):
            """Single-source projection. One PSUM bank per m-tile (interleaved
            accumulation groups must not share a bank: start=True clears the
            has-written bits bank-wide). evict2m(mj0, gm, ps_list).
            Yields once per 8-kt chunk (16 matmuls at gm=2).  First weight DMA
            issues eagerly at call time; later chunks prefetch one ahead."""
            nmt, nkt = M // 128, K // 128
            steps = [(mj0, kc0) for mj0 in range(0, nmt, 2)
                     for kc0 in range(0, nkt, 8)]
            wts = {}
            def load(i, split=False):
                mj0, kc0 = steps[i]
                kc = min(8, nkt - kc0)
                wt = T(wgp, [128, 8, 256], BF16, "wg")
                blk = w_d[:, (wcol0 + mj0 * 128) // 256, kc0 : kc0 + kc, :]
                if split and kc == 8:
                    nc.sync.dma_start(out=wt[:, 0:4, :], in_=blk[:, 0:4, :])
                    nc.sync.dma_start(out=wt[:, 4:8, :], in_=blk[:, 4:8, :])
                else:
                    nc.sync.dma_start(out=wt[:, :kc, :], in_=blk)
                wts[i] = wt
            load(0, split=True)
            def g():
                ps = None
                for i, (mj0, kc0) in enumerate(steps):
                    gm = min(2, nmt - mj0)
                    if kc0 == 0:
                        ps = [T(mmp, [128, 512], F32, "mm") for _ in range(gm)]
                    kc = min(8, nkt - kc0)
                    wt = wts.pop(i)
                    if i + 1 < len(steps):
                        load(i + 1)
                    for kt in range(kc0, kc0 + kc):
                        s = src(kt)
                        for mi in range(gm):
                            nc.tensor.matmul(
                                ps[mi][:, 0:256],
                                wt[:, kt - kc0, mi * 128 : (mi + 1) * 128], s,
                                start=(kt == 0), stop=(kt == nkt - 1))
                    if kc0 + 8 >= nkt:
                        evict2m(mj0, gm, ps)
                    yield
            return g()

        def ev_split(dsts_of_mj, btile, bcol_of_mj, func=AF.Identity):
            """projM eviction: per-mtile ACT evicts [128,256] with bias."""
            def _ev(mj0, gm, ps):
                for mi in range(gm):
                    nc.scalar.activation(
                        dsts_of_mj(mj0 + mi), ps[mi][:, 0:256],
                        func, bias=btile[:, bcol_of_mj(mj0 + mi)
                                         : bcol_of_mj(mj0 + mi) + 1])
            return _ev

        def scores_all(q_sl, k4):
            """psum [8,4,N]: row h of col-block e2 = q[h].k[e2,h] (q pre-scaled).
            q_sl [128,8,N] contiguous; k4 [128,4,8,N] e-major."""
            sp = T(scp, [8, 4, N], F32, "sc")
            for e2 in range(4):
                p = T(qkp, [128, 8, N], BF16, "qk")
                nc.vector.tensor_tensor(
                    out=p[:], in0=q_sl, in1=k4[:, e2, :, :], op=ALU.mult)
                for kt in range(8):
                    nc.tensor.matmul(sp[:, e2, :], o32[:, 32 - kt : 40 - kt],
                                     p[:, kt, :], start=(kt == 0), stop=(kt == 7))
            return sp

        def softmax_tiles(sp):
            """sp [8,4,N] psum scores -> 4 bf16 [8,N] attention-weight tiles."""
            et = T(smp, [8, 4, N], BF16, "sm")
            nc.scalar.activation(et[:], sp[:], AF.Exp)
            d = T(smdp, [8, N], F32, "smd")
            nc.vector.tensor_add(out=d[:], in0=et[:, 0, :], in1=et[:, 1, :])
            for e2 in (2, 3):
                nc.vector.tensor_add(out=d[:], in0=d[:], in1=et[:, e2, :])
            r = T(smdp, [8, N], F32, "smd")
            nc.vector.reciprocal_approx_fast(out=r[:], in_=d[:])
            outs = []
            for e2 in range(4):
                a = T(smbp, [8, N], BF16, "smb")
                nc.vector.tensor_tensor(out=a[:], in0=et[:, e2, :], in1=r[:],
                                        op=ALU.mult)
                outs.append(a)
            return outs

        def av_accum(a_list, v4, dst_sl):
            """dst_sl [128,8,N] contiguous = sum_e2 bcast(a_list[e2]) * V[e2].
            v4 [128,4,8,N] e-major.  All DVE; one scratch cur reused so the
            adds interleave with the next e2's mults."""
            for e2 in range(4):
                cur = dst_sl if e2 == 0 else T(qkp, [128, 8, N], BF16, "qk")
                for mt in range(0, 8, 2):
                    bp = T(brp, [128, 2, N], F32, "br")
                    for q in range(2):
                        nc.tensor.matmul(
                            bp[:, q, :],
                            sel[:, (mt + q) * 128 : (mt + q + 1) * 128],
                            a_list[e2][:], start=True, stop=True)
                    nc.vector.tensor_tensor(
                        out=cur[:, mt : mt + 2, :], in0=bp[:],
                        in1=v4[:, e2, mt : mt + 2, :], op=ALU.mult)
                if e2 > 0:
                    nc.vector.tensor_add(out=dst_sl, in0=dst_sl, in1=cur[:])

        def ln_norm(x_sl, g_t, b_t, dst_of_mt, feed=None):
            """LayerNorm over the 1024 feats of x_sl [128,8,N] (bf16, in-place
            scratch); writes normalized*g+b to dst_of_mt(mt).  Per-mt chain:
            2 DVE ops (all-bf16 SBUF, 2x mode) + 1 Scalar affine."""
            sq = T(qkp, [128, 8, N], BF16, "qk")
            nc.vector.tensor_tensor(out=sq[:], in0=x_sl, in1=x_sl, op=ALU.mult)
            st_s = T(brp, [1, N], F32, "br")
            for kt in range(8):
                nc.tensor.matmul(st_s[:], ones_b[:], x_sl[:, kt, :],
                                 start=(kt == 0), stop=(kt == 7))
            st_q = T(brp, [1, N], F32, "br")
            for kt in range(8):
                nc.tensor.matmul(st_q[:], ones_b[:], sq[:, kt, :],
                                 start=(kt == 0), stop=(kt == 7))
            mean = T(smrp, [1, N], F32, "smr")
            nc.vector.tensor_scalar_mul(mean[:], st_s[:], 1.0 / HD)
            mb = T(brp, [128, N], F32, "br")
            nc.tensor.matmul(mb[:], onerow_f[:], mean[:], start=True, stop=True)
            msq = T(smrp, [1, N], F32, "smr")
            nc.vector.tensor_scalar_mul(msq[:], st_q[:], 1.0 / HD)
            var = T(smrp, [1, N], F32, "smr")
            nc.vector.tensor_tensor(out=var[:], in0=mean[:], in1=mean[:],
                                    op=ALU.mult)
            nc.vector.tensor_tensor(out=var[:], in0=msq[:], in1=var[:],
                                    op=ALU.subtract)
            std = T(smrp, [1, N], F32, "smr")
            nc.scalar.activation(std[:], var[:], AF.Sqrt, bias=eps_t[:])
            rstd = T(smrp, [1, N], F32, "smr")
            nc.vector.reciprocal_approx_fast(out=rstd[:], in_=std[:])
            rb = T(brp, [128, N], F32, "br")
            nc.tensor.matmul(rb[:], onerow_f[:], rstd[:], start=True, stop=True)
            # bf16 SBUF copies of the broadcasts
            mbb = T(qkp, [128, 2, N], BF16, "qk")
            nc.scalar.activation(mbb[:, 0, :], mb[:], AF.Identity, bias=0.0)
            nc.scalar.activation(mbb[:, 1, :], rb[:], AF.Identity, bias=0.0)
            if feed is not None:
                feed.pump(1)
            for mt in range(8):
                nc.vector.tensor_tensor(out=x_sl[:, mt, :], in0=x_sl[:, mt, :],
                                        in1=mbb[:, 0, :], op=ALU.subtract)
                nc.vector.tensor_tensor(out=x_sl[:, mt, :], in0=x_sl[:, mt, :],
                                        in1=mbb[:, 1, :], op=ALU.mult)
                nc.scalar.activation(
                    dst_of_mt(mt), x_sl[:, mt, :], AF.Identity,
                    bias=b_t[:, mt : mt + 1], scale=g_t[:, mt : mt + 1])

        def gen_kv(mi, src4, k4, v4):
            """K/V projection of mha mi from src4 [128,8(kt),4(e),N] ->
            k4, v4 [128,4(e),8(mt),N] e-major."""
            def ev(mj, ps):
                dst = k4 if mj < 8 else v4
                bcol = 8 + mj      # k tiles: cols 8..15, v tiles: 16..23
                for p in range(2):
                    for q in range(2):
                        nc.scalar.activation(
                            dst[:, 2 * p + q, mj % 8, :],
                            ps[p][:, q * 256 : (q + 1) * 256],
                            AF.Identity,
                            bias=bqkv_t[mi][:, bcol : bcol + 1])
            return gen_projS(
                wqkv[mi], 2 * HD,
                lambda kt, p: src4[:, kt, 2 * p : 2 * p + 2, :], ev, wcol0=HD)

        def gen_q(mi, src4, qa):
            """q projection into a merged q/acc tile: scores consume the e1
            slice, then av_accum overwrites it with the AV result in place."""
            def ev(mj, ps):
                for p in range(2):
                    for q in range(2):
                        nc.scalar.activation(
                            qa[:, 2 * p + q, mj, :],
                            ps[p][:, q * 256 : (q + 1) * 256],
                            AF.Identity, bias=bqkv_t[mi][:, mj : mj + 1])
            return gen_projS(
                wqkv[mi], HD,
                lambda kt, p: src4[:, kt, 2 * p : 2 * p + 2, :], ev)

        def gen_expand(j, x_b, xp):
            # expand: m-tile m = e*8+mj -> xp[:, mj, e, :]
            def ev_exp(mj0, gm, ps):
                for mi in range(gm):
                    m = mj0 + mi
                    nc.scalar.activation(
                        xp[:, m % 8, m // 8, :], ps[mi][:, 0:256],
                        AF.Identity, bias=bexp_t[j][:, m : m + 1])
            return gen_projM(wexp[j], E * HD, HD,
                             lambda kt: x_b[:, kt, :], ev_exp)

        def gen_self_out(j, qa, xp, enh_dst):
            """out proj -> enh_dst pre-LN; residual added on DVE."""
            def ev_out(mj, ps):
                for p in range(2):
                    nc.scalar.activation(
                        enh_dst[:, mj, 2 * p : 2 * p + 2, :], ps[p][:],
                        AF.Identity, bias=bout_t[j][:, mj : mj + 1])
                nc.vector.tensor_tensor(
                    out=enh_dst[:, mj, :, :], in0=enh_dst[:, mj, :, :],
                    in1=xp[:, mj, :, :], op=ALU.add)
            return gen_projS(
                wout[j], HD,
                lambda kt, p: qa[:, 2 * p : 2 * p + 2, kt, :], ev_out)

        def attn_loop(qa, k4, v4, feed, fill=2):
            """self-attn e1 loop, software-pipelined; av result replaces q in
            qa in place one iteration late."""
            prev = None
            for e1 in range(4):
                sp = scores_all(qa[:, e1, :, :], k4)
                a_l = softmax_tiles(sp)
                feed.pump(fill)
                if prev is not None:
                    av_accum(prev, v4, qa[:, e1 - 1, :, :])
                prev = a_l
            feed.pump(fill)
            av_accum(prev, v4, qa[:, 3, :, :])

        def cross_loop(qa, k4, feed, fill=2):
            """cross-attn e1 loop accumulating abar (mean attn weights)."""
            abar = [None] * 4
            for e1 in range(4):
                sp = scores_all(qa[:, e1, :, :], k4)
                a_l = softmax_tiles(sp)
                feed.pump(fill)
                for e2 in range(4):
                    if e1 == 0:
                        ab = T(smabp, [8, N], BF16, "smab")
                        nc.vector.tensor_copy(out=ab[:], in_=a_l[e2][:])
                        abar[e2] = ab
                    else:
                        nc.vector.tensor_add(out=abar[e2][:], in0=abar[e2][:],
                                             in1=a_l[e2][:])
            return abar

        def cross_fin(mi, abar, v4, dst, feed=None):
            """abar-weighted AV + out proj (wout pre-scaled 0.25)."""
            cacc = T(accp, [128, 8, N], BF16, "big")
            av_accum(abar, v4, cacc[:])
            if feed is not None:
                feed.pump(6)
            drain(gen_projM(wout[mi], HD, HD, lambda kt: cacc[:, kt, :],
                            ev_split(lambda mj: dst[:, mj, :], bout_t[mi],
                                     lambda mj: mj)))

        def gen_gate(g, in_a, in_b, gt):
            return gen_projM(
                wgate[g], HD, 2 * HD,
                lambda kt: in_a[:, kt, :] if kt < 8 else in_b[:, kt - 8, :],
                ev_split(lambda mj: gt[:, mj, :], bgate_t[g],
                         lambda mj: mj, func=AF.Sigmoid))

        def run_selfB(j, enh_dst, sum_dst, feed):
            """LN each position of enh_dst in place; sum_dst = sum_e enh."""
            for e1 in range(4):
                ln_norm(enh_dst[:, :, e1, :], lng_t[j], lnb_t[j],
                        lambda mt, e1=e1: enh_dst[:, mt, e1, :], feed=feed)
                feed.pump(2)
            t2 = T(qkp, [128, 8, N], BF16, "qk")
            nc.vector.tensor_add(out=sum_dst[:], in0=enh_dst[:, :, 0, :],
                                 in1=enh_dst[:, :, 1, :])
            nc.vector.tensor_add(out=t2[:], in0=enh_dst[:, :, 2, :],
                                 in1=enh_dst[:, :, 3, :])
            nc.vector.tensor_add(out=sum_dst[:], in0=sum_dst[:], in1=t2[:])

        def gen_head(c, out):
            """Pass head: x load + expand(0) + kv0 + q0.  Run via a Feed so
            pass c's head can fill pass c-1's tail."""
            bs = slice(c * N, (c + 1) * N)
            xt_b = T(xbp, [128, 8, N], BF16, "xb")
            xs_b = T(xbp, [128, 8, N], BF16, "xb")
            for xd, xb in ((xt_d, xt_b), (xs_d, xs_b)):
                for h in range(4):
                    xf = T(qkp, [128, 2, N], F32, "qk")
                    nc.sync.dma_start(out=xf[:],
                                      in_=xd[:, 2 * h : 2 * h + 2, bs])
                    nc.vector.tensor_copy(out=xb[:, 2 * h : 2 * h + 2, :],
                                          in_=xf[:])
                yield
            xp_t = T(expp, [128, 8, 4, N], BF16, "exp")
            yield from gen_expand(0, xt_b, xp_t)
            k4t = T(qkvp, [128, 4, 8, N], BF16, "qkv")
            v4t = T(qkvp, [128, 4, 8, N], BF16, "qkv")
            yield from gen_kv(0, xp_t, k4t, v4t)
            qa_t = T(acc4p, [128, 4, 8, N], BF16, "acc4")
            yield from gen_q(0, xp_t, qa_t)
            out.update(xt_b=xt_b, xs_b=xs_b, xp_t=xp_t, k4t=k4t, v4t=v4t,
                       qa_t=qa_t)

        heads = [{} for _ in range(NP)]
        head_gens = [gen_head(c, heads[c]) for c in range(NP)]
        drain(head_gens[0])

        for c in range(NP):
            hd = heads[c]
            xt_b, xs_b, xp_t = hd["xt_b"], hd["xs_b"], hd["xp_t"]
            k4t, v4t, qa_t = hd["k4t"], hd["v4t"], hd["qa_t"]

            nc.mark("mid_start")
            t_enh = T(enhp, [128, 8, 4, N], BF16, "enh")
            s_enh = T(enhp, [128, 8, 4, N], BF16, "enh")
            sum_t = T(actp, [128, 8, N], BF16, "big")
            sum_s = T(actp, [128, 8, N], BF16, "big")

            # --- self-t, expand(1) fills the softmax windows
            xp_s = T(expp, [128, 8, 4, N], BF16, "exp")
            f = Feed(gen_expand(1, xs_b, xp_s))
            nc.mark("attn_t")
            attn_loop(qa_t, k4t, v4t, f, fill=3)
            k4s = T(qkvp, [128, 4, 8, N], BF16, "qkv")
            v4s = T(qkvp, [128, 4, 8, N], BF16, "qkv")
            gkv1 = gen_kv(1, xp_s, k4s, v4s)
            f.drain()
            # residual precompute frees xt_b/xs_b before the tail
            cres = (1.0 - res_w) * 0.5
            osum = T(xbp, [128, 8, N], BF16, "xb")
            nc.vector.tensor_add(out=osum[:], in0=xt_b[:], in1=xs_b[:])
            nc.vector.tensor_scalar_mul(osum[:], osum[:], cres)
            # --- kv1 stream, then self-out t
            nc.mark("kv1")
            drain(gkv1)
            nc.mark("self_out0")
            drain(gen_self_out(0, qa_t, xp_t, t_enh))
            # --- q1 with t-LN interleaved
            qa_s = T(acc4p, [128, 4, 8, N], BF16, "acc4")
            f = Feed(gen_q(1, xp_s, qa_s))
            nc.mark("q1_selfB0")
            run_selfB(0, t_enh, sum_t, f)
            f.drain()
            # --- self-s, q2 fills the windows
            qa_c2 = T(acc4p, [128, 4, 8, N], BF16, "acc4")
            f = Feed(gen_q(2, t_enh, qa_c2))
            nc.mark("attn_s")
            attn_loop(qa_s, k4s, v4s, f, fill=2)
            f.drain()
            # --- kv3 chunks cover the last av's adds before self-out s
            k4c3 = T(qkvp, [128, 4, 8, N], BF16, "qkv")
            v4c3 = T(qkvp, [128, 4, 8, N], BF16, "qkv")
            f = Feed(gen_kv(3, t_enh, k4c3, v4c3))
            f.pump(2)
            nc.mark("self_out1")
            drain(gen_self_out(1, qa_s, xp_s, s_enh))
            nc.mark("kv3_selfB1")
            run_selfB(1, s_enh, sum_s, f)
            f.drain()
            # --- q3 stream
            qa_c3 = T(acc4p, [128, 4, 8, N], BF16, "acc4")
            nc.mark("q3")
            drain(gen_q(3, s_enh, qa_c3))
            # --- cross st (mha3): joint/vj/qj fill the windows
            joint = T(actp, [128, 8, N], BF16, "big")
            vj = T(actp, [128, 8, N], BF16, "big")
            qj = T(actp, [128, 8, N], BF16, "big")
            mst = T(meanp, [128, 8, N], BF16, "big")
            mts = T(meanp, [128, 8, N], BF16, "big")
            f = Feed(
                gen_projM(wjoint, HD, 2 * HD,
                          lambda kt: sum_t[:, kt, :] if kt < 8
                          else sum_s[:, kt - 8, :],
                          ev_split(lambda mj: joint[:, mj, :], bjoint_t,
                                   lambda mj: mj)),
                lambda: gen_projM(wqkv[4], HD, HD, lambda kt: joint[:, kt, :],
                                  ev_split(lambda mj: vj[:, mj, :], bqkv_t[4],
                                           lambda mj: 16 + mj),
                                  wcol0=2 * HD),
                lambda: gen_projM(wqkv[4], HD, HD, lambda kt: joint[:, kt, :],
                                  ev_split(lambda mj: qj[:, mj, :], bqkv_t[4],
                                           lambda mj: mj)))
            nc.mark("cross_loop3")
            abar3 = cross_loop(qa_c3, k4c3, f, fill=3)
            f.drain()
            # --- finish cross st; kv2 chunks fill the av window
            k4c2 = T(qkvp, [128, 4, 8, N], BF16, "qkv")
            v4c2 = T(qkvp, [128, 4, 8, N], BF16, "qkv")
            fkv2 = Feed(gen_kv(2, s_enh, k4c2, v4c2))
            nc.mark("crossfin3_kv2")
            cross_fin(3, abar3, v4c3, mst, feed=fkv2)
            fkv2.drain()
            # --- cross ts (mha2): mtj fills the windows
            mtj = T(meanp, [128, 8, N], BF16, "big")
            f = Feed(gen_projM(wout[4], HD, HD, lambda kt: vj[:, kt, :],
                               ev_split(lambda mj: mtj[:, mj, :], bout_t[4],
                                        lambda mj: mj)))
            nc.mark("cross_loop2_mtj")
            abar2 = cross_loop(qa_c2, k4c2, f, fill=1)
            f.drain()
            # --- finish cross ts; kv4t chunks fill the av window
            k4j1 = T(qkvp, [128, 4, 8, N], BF16, "qkv")
            v4j1 = T(qkvp, [128, 4, 8, N], BF16, "qkv")
            fkv4 = Feed(gen_kv(4, t_enh, k4j1, v4j1))
            nc.mark("crossfin2_kv4t")
            cross_fin(2, abar2, v4c2, mts, feed=fkv4)
            fkv4.drain()
            gate_t = T(gatep, [128, 8, N], BF16, "big")
            g0 = Feed(gen_gate(0, mts, mtj, gate_t))
            # --- jx: single-query cross-attn (q = joint row)
            jacc = T(acc4p, [128, 4, 8, N], BF16, "acc4")
            k4j2 = T(qkvp, [128, 4, 8, N], BF16, "qkv")
            v4j2 = T(qkvp, [128, 4, 8, N], BF16, "qkv")
            fkv = Feed(gen_kv(4, s_enh, k4j2, v4j2))
            nxt = Feed()
            if c + 1 < NP:
                nxt.add(head_gens[c + 1])
            nc.mark("jx1")
            a_l1 = softmax_tiles(scores_all(qj[:], k4j1))
            fkv.pump(4)
            g0.pump(2)
            av_accum(a_l1, v4j1, jacc[:, 0, :, :])
            nxt.pump(2)
            fkv.drain()
            nc.mark("jx2")
            a_l2 = softmax_tiles(scores_all(qj[:], k4j2))
            g0.drain()
            nxt.pump(2)
            # gate_t consumed immediately so gate_s can reuse the pool slot
            f2 = T(accp, [128, 8, N], BF16, "big")
            nc.gpsimd.tensor_tensor(out=f2[:], in0=gate_t[:], in1=mtj[:],
                                    op=ALU.mult)
            nc.gpsimd.tensor_tensor(out=mts[:], in0=gate_t[:], in1=mts[:],
                                    op=ALU.mult)
            gate_s = T(gatep, [128, 8, N], BF16, "big")
            g1 = Feed(gen_gate(1, mst, mtj, gate_s))
            g1.pump(3)
            av_accum(a_l2, v4j2, jacc[:, 1, :, :])
            g1.pump(3)
            mjt = T(meanp, [128, 8, N], BF16, "big")
            mjs = T(meanp, [128, 8, N], BF16, "big")
            def ev_jx(mj, ps):
                for jj, dst in enumerate((mjt, mjs)):
                    nc.scalar.activation(
                        dst[:, mj, :], ps[0][:, jj * 256 : (jj + 1) * 256],
                        AF.Identity, bias=bout_t[4][:, mj : mj + 1])
            nc.mark("evjx")
            drain(gen_projS(wout[4], HD, lambda kt, p: jacc[:, 0:2, kt, :],
                            ev_jx, npair=1))
            g1.drain()
            nxt.pump(3)
            nc.gpsimd.tensor_tensor(out=mst[:], in0=gate_s[:], in1=mst[:],
                                    op=ALU.mult)
            nc.gpsimd.tensor_tensor(out=mtj[:], in0=gate_s[:], in1=mtj[:],
                                    op=ALU.mult)
            gate_j = T(gatep, [128, 8, N], BF16, "big")
            nc.mark("gate2")
            nxt.pump(2)
            drain(gen_gate(2, mjt, mjs, gate_j))
            nc.vector.tensor_tensor(out=mjt[:], in0=gate_j[:], in1=mjt[:],
                                    op=ALU.mult)
            nc.vector.tensor_tensor(out=mjs[:], in0=gate_j[:], in1=mjs[:],
                                    op=ALU.mult)
            fs = [mts, mst, f2, mtj, mjt, mjs]

            # --- tail: wo1/wo2/final LN, next pass's head keeps pumping
            nc.mark("tail_wo1")
            nxt.pump(3)
            h1 = T(expp, [128, 8, 4, N], BF16, "exp")
            def ev_h1(mj0, gm, ps):
                for mi in range(gm):
                    m = mj0 + mi
                    nc.scalar.activation(
                        h1[:, m % 8, m // 8, :], ps[mi][:, 0:256],
                        AF.Relu, bias=bo1_t[:, m : m + 1])
            g = gen_projM(wo1, 2 * HD, 6 * HD,
                          lambda kt: fs[kt // 8][:, kt % 8, :], ev_h1)
            i = 0
            for _ in g:
                i += 1
                if i % 4 == 0:
                    nxt.pump(1)
            nc.mark("wo2")
            h2 = T(accp, [128, 8, N], BF16, "big")
            nxt.pump(2)
            g = gen_projM(wo2, HD, 2 * HD,
                          lambda kt: h1[:, kt % 8, kt // 8, :],
                          ev_split(lambda mj: h2[:, mj, :], bo2_t,
                                   lambda mj: mj))
            for _ in g:
                nxt.pump(1)

            # final LN (g,b pre-scaled by res_w) + (1-res_w)/2*(xt+xs)
            nc.mark("final_ln")
            yt = T(qkp, [128, 8, N], BF16, "qk")
            ln_norm(h2[:], lng_t[2], lnb_t[2], lambda mt: yt[:, mt, :],
                    feed=nxt)
            nxt.pump(4)
            nc.vector.tensor_add(out=yt[:], in0=yt[:], in1=osum[:])
            nc.sync.dma_start(out=y_d[:, :, slice(c * N, (c + 1) * N)],
                              in_=yt[:])
            nxt.drain()

    nc.compile()
    return nc


def _sel_const():
    s = np.zeros((8, 8 * 128), np.float32)
    for mt in range(8):
        s[mt, mt * 128 : (mt + 1) * 128] = 1.0
    return s.astype(BF)


def _o32_const():
    o = np.zeros((128, 64), np.float32)
    o[:, 32] = 1.0
    return o.astype(BF)


def _wl(w):
    """torch-style [M_out, K_in] -> pair-blocked [128, M/256, K/128, 256] bf16
    (one 2-mtile all-kt block contiguous per partition)."""
    a = np.asarray(w, np.float32).T          # [K, M]
    K, M = a.shape
    a = a.reshape(K // 128, 128, M // 256, 256).transpose(1, 2, 0, 3)
    return np.ascontiguousarray(a).astype(BF)


def _prep_inputs(i):
    res_w = float(np.asarray(i["res_w"]).reshape(-1)[0])
    sc = 1.0 / math.sqrt(128.0)

    shared = {
        "wexp0": _wl(i["exp_t_w"]), "wexp1": _wl(i["exp_s_w"]),
        "bexp0": _bias_cols(np.asarray(i["exp_t_b"]) + np.asarray(i["pos_enc"]).reshape(-1)),
        "bexp1": _bias_cols(np.asarray(i["exp_s_b"]) + np.asarray(i["pos_enc"]).reshape(-1)),
        "wjoint": _wl(np.asarray(i["joint_w"], np.float32) * 0.25),
        "bjoint": _bias_cols(i["joint_b"]),
        "wo1": _wl(i["out1_w"]), "bo1": _bias_cols(i["out1_b"]),
        "wo2": _wl(i["out2_w"]), "bo2": _bias_cols(i["out2_b"]),
        "sel_c": _sel_const(), "o32_c": _o32_const(),
    }
    for g in range(3):
        shared[f"wgate{g}"] = _wl(i["gate_w"][g])
        shared[f"bgate{g}"] = _bias_cols(i["gate_b"][g])
    for m in range(5):
        w = np.asarray(i["mha_in_w"][m], np.float32).copy()
        b = np.asarray(i["mha_in_b"][m], np.float32).copy()
        w[:HD] *= sc
        b[:HD] *= sc
        shared[f"wqkv{m}"] = _wl(w)
        shared[f"bqkv{m}"] = _bias_cols(b)
        wo = np.asarray(i["mha_out_w"][m], np.float32)
        if m in (2, 3):
            wo = wo * 0.25      # fold mean over the 4 query positions
        shared[f"wout{m}"] = _wl(wo)
        shared[f"bout{m}"] = _bias_cols(i["mha_out_b"][m])
    for ln in range(3):
        g = np.asarray(i["ln_g"][ln], np.float32)
        b = np.asarray(i["ln_b"][ln], np.float32)
        if ln == 2:
            g = g * res_w
            b = b * res_w
        shared[f"lng{ln}"] = _bias_cols(g)
        shared[f"lnb{ln}"] = _bias_cols(b)

    def shard_x(x, c):
        xc = np.asarray(x, np.float32)[c * BC : (c + 1) * BC, 0, :]  # [512,1024]
        return np.ascontiguousarray(xc.T.reshape(8, 128, BC).transpose(1, 0, 2))

    in_maps = []
    for c in range(NCORES):
        m = dict(shared)
        m["xt"] = shard_x(i["temporal_features"], c)
        m["xs"] = shard_x(i["spatial_features"], c)
        in_maps.append(m)
    return res_w, in_maps


def kernel(**inputs):
    res_w, in_maps = _prep_inputs(inputs)
    nc = build(res_w)
    res = bass_utils.run_bass_kernel_spmd(nc, in_maps, core_ids=list(range(NCORES)))
    outs = []
    for c in range(NCORES):
        y = res.results[c]["y"]                                   # [128,8,512]
        outs.append(np.asarray(y).astype(np.float32)
                    .transpose(1, 0, 2).reshape(HD, BC).T)
    return np.concatenate(outs, 0)[:, None, :].astype(np.float32)


# revision 24
# speedup vs baseline: 1.0378x; 1.0378x over previous
"""Trainium2 Bass kernel for nn_MISA (dense_transformer, data-parallel over 8 cores).

Layout: feature-major activations [feat_part=128, mtile, batch_cols] per core.
Batch 4096 -> 512 per core -> two passes of 256 columns.
All matmuls bf16 (fp32 PSUM accumulation); LN/softmax internals fp32.

v3 (PE-gap removal): the PE is the bottleneck engine; v2 lost ~400us to PE
idle gaps waiting on DVE softmax/LN chains plus ~300us of HAM cold-throttle
restarts those gaps caused.  v3 keeps the PE warm:
- every projection is a generator yielding at m-tile boundaries; a Feed
  object interleaves pending projection chunks into every DVE-bound window
  (softmax, LayerNorm, gate chains) as PE filler.
- attention loops are software-pipelined: scores(e1) / softmax(e1) /
  filler / av(e1-1), so av's dependency on softmax is always satisfied by
  the time the PE reaches it.
- av accumulation adds run on GpSimd (otherwise idle), LN's per-tile
  affine (g,b) runs on the Scalar engine, self-attn residual adds moved
  from PE identity-matmuls to DVE.
- pass c+1's head (x load, expand, kv0, q0) is pumped as PE filler during
  pass c's tail (wo1/wo2/final LN), removing the inter-pass PE gap.

Structural simplifications (exact, not approximations):
- attention with all-equal keys/values (q/k/v = broadcast joint row) is the
  identity on v: cross_tj == cross_sj == out_proj4(v_proj4(joint)).
- mean over query positions commutes with out_proj and with A@V, so the six
  cross outputs never materialize per-query outputs (abar-weighted V only).
- all-equal queries (j as q): single query row, output equals its mean.
"""
import sys, math
from contextlib import ExitStack
sys.path.insert(0, "/opt/trn_rl_repo")

import numpy as np
import ml_dtypes

import concourse.bass as bass
import concourse.mybir as mybir
from concourse import bacc
import concourse.tile as tile
from concourse import bass_utils

F32 = mybir.dt.float32
BF16 = mybir.dt.bfloat16
AF = mybir.ActivationFunctionType
ALU = mybir.AluOpType
BF = ml_dtypes.bfloat16

H = 8
E = 4
HD = 1024
B = 4096
NCORES = 8
BC = B // NCORES          # 512 batch per core
NP = 2                    # passes per core
N = BC // NP              # 256 batch cols per pass
EPS = 1e-5


def _bias_cols(b):
    # [M] -> [128, M//128]: column m = per-partition bias of m-tile m
    return np.ascontiguousarray(np.asarray(b, np.float32).reshape(-1, 128).T)


class Feed:
    """Queue of projection generators (or factories) pumped as PE filler.
    Factories instantiate lazily when reached, so their eager first weight
    DMA overlaps the tail of the preceding stream."""
    def __init__(self, *gens):
        self.q = list(gens)

    def add(self, g):
        self.q.append(g)

    def pump(self, n=1):
        while n > 0 and self.q:
            g = self.q[0]
            if not hasattr(g, "__next__"):
                g = self.q[0] = g()
            try:
                next(g)
                n -= 1
            except StopIteration:
                self.q.pop(0)

    def drain(self):
        while self.q:
            self.pump(1)


def drain(g):
    for _ in g:
        pass


def build(res_w: float):
    nc = bacc.Bacc("TRN2", target_bir_lowering=False, debug=False)

    def din(name, shape, dt):
        return nc.dram_tensor(name, list(shape), dt, kind="ExternalInput").ap()

    xt_d = din("xt", (128, 8, BC), F32)
    xs_d = din("xs", (128, 8, BC), F32)
    # weights pair-blocked: [128, M/256, K/128, 256] — one (2-mtile, all-kt)
    # block is contiguous per partition, so block DMAs run at full rate
    wexp = [din(f"wexp{j}", (128, 16, 8, 256), BF16) for j in range(2)]
    bexp = [din(f"bexp{j}", (128, 32), F32) for j in range(2)]
    wqkv = [din(f"wqkv{i}", (128, 12, 8, 256), BF16) for i in range(5)]
    bqkv = [din(f"bqkv{i}", (128, 24), F32) for i in range(5)]
    wout = [din(f"wout{i}", (128, 4, 8, 256), BF16) for i in range(5)]
    bout = [din(f"bout{i}", (128, 8), F32) for i in range(5)]
    wjoint = din("wjoint", (128, 4, 16, 256), BF16)
    bjoint = din("bjoint", (128, 8), F32)
    wgate = [din(f"wgate{g}", (128, 4, 16, 256), BF16) for g in range(3)]
    bgate = [din(f"bgate{g}", (128, 8), F32) for g in range(3)]
    wo1 = din("wo1", (128, 8, 48, 256), BF16)
    bo1 = din("bo1", (128, 16), F32)
    wo2 = din("wo2", (128, 4, 16, 256), BF16)
    bo2 = din("bo2", (128, 8), F32)
    lng = [din(f"lng{i}", (128, 8), F32) for i in range(3)]
    lnb = [din(f"lnb{i}", (128, 8), F32) for i in range(3)]
    sel_d = din("sel_c", (8, 8 * 128), BF16)
    o32_d = din("o32_c", (128, 64), BF16)
    y_d = nc.dram_tensor("y", [128, 8, BC], BF16, kind="ExternalOutput").ap()

    with tile.TileContext(nc) as tc, ExitStack() as ctx:
        P = lambda **kw: ctx.enter_context(tc.tile_pool(**kw))
        cst = P(name="cst", bufs=1)
        wgp = P(name="wgp", bufs=3)                 # [128,8,256] weight blocks
        mmp = P(name="mmp", bufs=4, space="PSUM")   # 4 x [128,512] banks
        scp = P(name="scp", bufs=1, space="PSUM")   # [8,4,N] = 2 banks
        brp = P(name="brp", bufs=2, space="PSUM")   # 2 x [128,N] banks
        expp = P(name="expp", bufs=2)   # xp_t, xp_s; also h1, next xp_t
        enhp = P(name="enhp", bufs=2)   # t_enh, s_enh [128,8,4,N]
        qkvp = P(name="qkvp", bufs=2)   # k4, v4 [128,4,8,N]
        acc4p = P(name="acc4p", bufs=2)  # qa tiles + jacc [128,4,8,N]
        bigp = P(name="bigp", bufs=7)   # all long-lived [128,8,N] bf16
        actp = meanp = accp = gatep = bigp
        qkp = P(name="qkp", bufs=2)     # qk products, av curs, ln scratch
        xbp = P(name="xbp", bufs=3)     # xt_b, xs_b, osum
        smp = P(name="smp", bufs=1)     # softmax exp [8,4,N] bf16
        smdp = P(name="smdp", bufs=2)   # softmax denom [8,N] f32
        smbp = P(name="smbp", bufs=3)   # a_l bf16 [8,N]
        smrp = P(name="smrp", bufs=3)   # LN scalars [1,N] f32
        smabp = P(name="smabp", bufs=4)  # abar f32 [8,N]

        nc._phase_marks = []
        def mark(name):
            nc._phase_marks.append((name, nc.next_id()))
        nc.mark = mark

        _tc = [0]
        def T(pool, shape, dtype, tag):
            _tc[0] += 1
            return pool.tile(shape, dtype, tag=tag, name=f"{tag}_{_tc[0]}")

        ones_b = T(cst, [128, 1], BF16, "ones_b")
        nc.any.memset(ones_b[:], 1.0)
        onerow_f = T(cst, [1, 128], F32, "onerow_f")
        nc.any.memset(onerow_f[:], 1.0)
        sel = T(cst, [8, 8 * 128], BF16, "sel")
        nc.sync.dma_start(out=sel[:], in_=sel_d)
        o32 = T(cst, [128, 64], BF16, "o32")   # all-ones column at col 32
        nc.sync.dma_start(out=o32[:], in_=o32_d)
        eps_t = T(cst, [1, 1], F32, "eps_t")
        nc.any.memset(eps_t[:], EPS)

        def ctile(name, ap):
            t = cst.tile(list(ap.shape), ap.dtype, tag=name)
            nc.sync.dma_start(out=t[:], in_=ap)
            return t

        bexp_t = [ctile(f"bexp{j}", bexp[j]) for j in range(2)]
        bqkv_t = [ctile(f"bqkv{i}", bqkv[i]) for i in range(5)]
        bout_t = [ctile(f"bout{i}", bout[i]) for i in range(5)]
        bjoint_t = ctile("bjoint", bjoint)
        bgate_t = [ctile(f"bgate{g}", bgate[g]) for g in range(3)]
        bo1_t = ctile("bo1", bo1)
        bo2_t = ctile("bo2", bo2)
        lng_t = [ctile(f"lng{i}", lng[i]) for i in range(3)]
        lnb_t = [ctile(f"lnb{i}", lnb[i]) for i in range(3)]

        def gen_projS(w_d, M, src_pair, evict2, wcol0=0, npair=2):
            """Shared-weight projection, K=1024. src_pair(kt, p) -> [128,2,N]
            moving pair; two pairs (4 sources) per stationary load.
            evict2(mj, psums): psums[p] = [128,512] = pair p's two outputs.
            Yields once per mj (16 matmuls).  The first weight DMA issues
            eagerly at call time; later blocks prefetch one group ahead."""
            nmt = M // 128
            blocks = list(range(0, nmt, 2))
            wts = {}
            def load(mj0, split=False):
                wt = T(wgp, [128, 8, 256], BF16, "wg")
                blk = w_d[:, (wcol0 + mj0 * 128) // 256, :, :]
                if split:
                    nc.sync.dma_start(out=wt[:, 0:4, :], in_=blk[:, 0:4, :])
                    nc.sync.dma_start(out=wt[:, 4:8, :], in_=blk[:, 4:8, :])
                else:
                    nc.sync.dma_start(out=wt[:], in_=blk)
                wts[mj0] = wt
            load(blocks[0], split=True)
            def g():
                for bi, mj0 in enumerate(blocks):
                    gm = min(2, nmt - mj0)
                    wt = wts.pop(mj0)
                    for mj in range(mj0, mj0 + gm):
                        ps = [T(mmp, [128, 512], F32, "mm")
                              for _ in range(npair)]
                        for kt in range(8):
                            w_sl = wt[:, kt,
                                      (mj - mj0) * 128 : (mj - mj0 + 1) * 128]
                            for p in range(npair):
                                nc.tensor.matmul(ps[p][:], w_sl,
                                                 src_pair(kt, p),
                                                 start=(kt == 0),
                                                 stop=(kt == 7))
                        evict2(mj, ps)
                        if mj == mj0 and bi + 1 < len(blocks):
                            load(blocks[bi + 1])
                        yield
            return g()

        def gen_projM(w_d, M, K, src, evict2m, wcol0=0):
            """Single-source projection. One PSUM bank per m-tile (interleaved
            accumulation groups must not share a bank: start=True clears the
            has-written bits bank-wide). evict2m(mj0, gm, ps_list).
            Yields once per 8-kt chunk (16 matmuls at gm=2).  First weight DMA
            issues eagerly at call time; later chunks prefetch one ahead."""
            nmt, nkt = M // 128, K // 128
            steps = [(mj0, kc0) for mj0 in range(0, nmt, 2)
                     for kc0 in range(0, nkt, 8)]
            wts = {}
            def load(i, split=False):
                mj0, kc0 = steps[i]
                kc = min(8, nkt - kc0)
                wt = T(wgp, [128, 8, 256], BF16, "wg")
                blk = w_d[:, (wcol0 + mj0 * 128) // 256, kc0 : kc0 + kc, :]
                if split and kc == 8:
                    nc.sync.dma_start(out=wt[:, 0:4, :], in_=blk[:, 0:4, :])
                    nc.sync.dma_start(out=wt[:, 4:8, :], in_=blk[:, 4:8, :])
                else:
                    nc.sync.dma_start(out=wt[:, :kc, :], in_=blk)
                wts[i] = wt
            load(0, split=True)
            def g():
                ps = None
                for i, (mj0, kc0) in enumerate(steps):
                    gm = min(2, nmt - mj0)
                    if kc0 == 0:
                        ps = [T(mmp, [128, 512], F32, "mm") for _ in range(gm)]
                    kc = min(8, nkt - kc0)
                    wt = wts.pop(i)
                    if i + 1 < len(steps):
                        load(i + 1)
                    for kt in range(kc0, kc0 + kc):
                        s = src(kt)
                        for mi in range(gm):
                            nc.tensor.matmul(
                                ps[mi][:, 0:256],
                                wt[:, kt - kc0, mi * 128 : (mi + 1) * 128], s,
                                start=(kt == 0), stop=(kt == nkt - 1))
                    if kc0 + 8 >= nkt:
                        evict2m(mj0, gm, ps)
                    yield
            return g()

        def ev_split(dsts_of_mj, btile, bcol_of_mj, func=AF.Identity):
            """projM eviction: per-mtile ACT evicts [128,256] with bias."""
            def _ev(mj0, gm, ps):
                for mi in range(gm):
                    nc.scalar.activation(
                        dsts_of_mj(mj0 + mi), ps[mi][:, 0:256],
                        func, bias=btile[:, bcol_of_mj(mj0 + mi)
                                         : bcol_of_mj(mj0 + mi) + 1])
            return _ev

        def scores_all(q_sl, k4):
            """psum [8,4,N]: row h of col-block e2 = q[h].k[e2,h] (q pre-scaled).
            q_sl [128,8,N] contiguous; k4 [128,4,8,N] e-major."""
            sp = T(scp, [8, 4, N], F32, "sc")
            for e2 in range(4):
                p = T(qkp, [128, 8, N], BF16, "qk")
                nc.vector.tensor_tensor(
                    out=p[:], in0=q_sl, in1=k4[:, e2, :, :], op=ALU.mult)
                for kt in range(8):
                    nc.tensor.matmul(sp[:, e2, :], o32[:, 32 - kt : 40 - kt],
                                     p[:, kt, :], start=(kt == 0), stop=(kt == 7))
            return sp

        def softmax_tiles(sp):
            """sp [8,4,N] psum scores -> 4 bf16 [8,N] attention-weight tiles."""
            et = T(smp, [8, 4, N], BF16, "sm")
            nc.scalar.activation(et[:], sp[:], AF.Exp)
            d = T(smdp, [8, N], F32, "smd")
            nc.vector.tensor_add(out=d[:], in0=et[:, 0, :], in1=et[:, 1, :])
            for e2 in (2, 3):
                nc.vector.tensor_add(out=d[:], in0=d[:], in1=et[:, e2, :])
            r = T(smdp, [8, N], F32, "smd")
            nc.vector.reciprocal_approx_fast(out=r[:], in_=d[:])
            outs = []
            for e2 in range(4):
                a = T(smbp, [8, N], BF16, "smb")
                nc.vector.tensor_tensor(out=a[:], in0=et[:, e2, :], in1=r[:],
                                        op=ALU.mult)
                outs.append(a)
            return outs

        def av_accum(a_list, v4, dst_sl):
            """dst_sl [128,8,N] contiguous = sum_e2 bcast(a_list[e2]) * V[e2].
            v4 [128,4,8,N] e-major.  All DVE; one scratch cur reused so the
            adds interleave with the next e2's mults."""
            for e2 in range(4):
                cur = dst_sl if e2 == 0 else T(qkp, [128, 8, N], BF16, "qk")
                for mt in range(0, 8, 2):
                    bp = T(brp, [128, 2, N], F32, "br")
                    for q in range(2):
                        nc.tensor.matmul(
                            bp[:, q, :],
                            sel[:, (mt + q) * 128 : (mt + q + 1) * 128],
                            a_list[e2][:], start=True, stop=True)
                    nc.vector.tensor_tensor(
                        out=cur[:, mt : mt + 2, :], in0=bp[:],
                        in1=v4[:, e2, mt : mt + 2, :], op=ALU.mult)
                if e2 > 0:
                    nc.vector.tensor_add(out=dst_sl, in0=dst_sl, in1=cur[:])

        def ln_norm(x_sl, g_t, b_t, dst_of_mt, feed=None):
            """LayerNorm over the 1024 feats of x_sl [128,8,N] (bf16, in-place
            scratch); writes normalized*g+b to dst_of_mt(mt).  Per-mt chain:
            2 DVE ops (all-bf16 SBUF, 2x mode) + 1 Scalar affine."""
            sq = T(qkp, [128, 8, N], BF16, "qk")
            nc.vector.tensor_tensor(out=sq[:], in0=x_sl, in1=x_sl, op=ALU.mult)
            st_s = T(brp, [1, N], F32, "br")
            for kt in range(8):
                nc.tensor.matmul(st_s[:], ones_b[:], x_sl[:, kt, :],
                                 start=(kt == 0), stop=(kt == 7))
            st_q = T(brp, [1, N], F32, "br")
            for kt in range(8):
                nc.tensor.matmul(st_q[:], ones_b[:], sq[:, kt, :],
                                 start=(kt == 0), stop=(kt == 7))
            mean = T(smrp, [1, N], F32, "smr")
            nc.vector.tensor_scalar_mul(mean[:], st_s[:], 1.0 / HD)
            mb = T(brp, [128, N], F32, "br")
            nc.tensor.matmul(mb[:], onerow_f[:], mean[:], start=True, stop=True)
            msq = T(smrp, [1, N], F32, "smr")
            nc.vector.tensor_scalar_mul(msq[:], st_q[:], 1.0 / HD)
            var = T(smrp, [1, N], F32, "smr")
            nc.vector.tensor_tensor(out=var[:], in0=mean[:], in1=mean[:],
                                    op=ALU.mult)
            nc.vector.tensor_tensor(out=var[:], in0=msq[:], in1=var[:],
                                    op=ALU.subtract)
            std = T(smrp, [1, N], F32, "smr")
            nc.scalar.activation(std[:], var[:], AF.Sqrt, bias=eps_t[:])
            rstd = T(smrp, [1, N], F32, "smr")
            nc.vector.reciprocal_approx_fast(out=rstd[:], in_=std[:])
            rb = T(brp, [128, N], F32, "br")
            nc.tensor.matmul(rb[:], onerow_f[:], rstd[:], start=True, stop=True)
            # bf16 SBUF copies of the broadcasts
            mbb = T(qkp, [128, 2, N], BF16, "qk")
            nc.scalar.activation(mbb[:, 0, :], mb[:], AF.Identity, bias=0.0)
            nc.scalar.activation(mbb[:, 1, :], rb[:], AF.Identity, bias=0.0)
            if feed is not None:
                feed.pump(1)
            for mt in range(8):
                nc.vector.tensor_tensor(out=x_sl[:, mt, :], in0=x_sl[:, mt, :],
                                        in1=mbb[:, 0, :], op=ALU.subtract)
                nc.vector.tensor_tensor(out=x_sl[:, mt, :], in0=x_sl[:, mt, :],
                                        in1=mbb[:, 1, :], op=ALU.mult)
                nc.scalar.activation(
                    dst_of_mt(mt), x_sl[:, mt, :], AF.Identity,
                    bias=b_t[:, mt : mt + 1], scale=g_t[:, mt : mt + 1])

        def gen_kv(mi, src4, k4, v4):
            """K/V projection of mha mi from src4 [128,8(kt),4(e),N] ->
            k4, v4 [128,4(e),8(mt),N] e-major."""
            def ev(mj, ps):
                dst = k4 if mj < 8 else v4
                bcol = 8 + mj      # k tiles: cols 8..15, v tiles: 16..23
                for p in range(2):
                    for q in range(2):
                        nc.scalar.activation(
                            dst[:, 2 * p + q, mj % 8, :],
                            ps[p][:, q * 256 : (q + 1) * 256],
                            AF.Identity,
                            bias=bqkv_t[mi][:, bcol : bcol + 1])
            return gen_projS(
                wqkv[mi], 2 * HD,
                lambda kt, p: src4[:, kt, 2 * p : 2 * p + 2, :], ev, wcol0=HD)

        def gen_q(mi, src4, qa):
            """q projection into a merged q/acc tile: scores consume the e1
            slice, then av_accum overwrites it with the AV result in place."""
            def ev(mj, ps):
                for p in range(2):
                    for q in range(2):
                        nc.scalar.activation(
                            qa[:, 2 * p + q, mj, :],
                            ps[p][:, q * 256 : (q + 1) * 256],
                            AF.Identity, bias=bqkv_t[mi][:, mj : mj + 1])
            return gen_projS(
                wqkv[mi], HD,
                lambda kt, p: src4[:, kt, 2 * p : 2 * p + 2, :], ev)

        def gen_expand(j, x_b, xp):
            # expand: m-tile m = e*8+mj -> xp[:, mj, e, :]
            def ev_exp(mj0, gm, ps):
                for mi in range(gm):
                    m = mj0 + mi
                    nc.scalar.activation(
                        xp[:, m % 8, m // 8, :], ps[mi][:, 0:256],
                        AF.Identity, bias=bexp_t[j][:, m : m + 1])
            return gen_projM(wexp[j], E * HD, HD,
                             lambda kt: x_b[:, kt, :], ev_exp)

        def gen_self_out(j, qa, xp, enh_dst):
            """out proj -> enh_dst pre-LN; residual added on DVE."""
            def ev_out(mj, ps):
                for p in range(2):
                    nc.scalar.activation(
                        enh_dst[:, mj, 2 * p : 2 * p + 2, :], ps[p][:],
                        AF.Identity, bias=bout_t[j][:, mj : mj + 1])
                nc.vector.tensor_tensor(
                    out=enh_dst[:, mj, :, :], in0=enh_dst[:, mj, :, :],
                    in1=xp[:, mj, :, :], op=ALU.add)
            return gen_projS(
                wout[j], HD,
                lambda kt, p: qa[:, 2 * p : 2 * p + 2, kt, :], ev_out)

        def attn_loop(qa, k4, v4, feed, fill=2):
            """self-attn e1 loop, software-pipelined; av result replaces q in
            qa in place one iteration late."""
            prev = None
            for e1 in range(4):
                sp = scores_all(qa[:, e1, :, :], k4)
                a_l = softmax_tiles(sp)
                feed.pump(fill)
                if prev is not None:
                    av_accum(prev, v4, qa[:, e1 - 1, :, :])
                prev = a_l
            feed.pump(fill)
            av_accum(prev, v4, qa[:, 3, :, :])

        def cross_loop(qa, k4, feed, fill=2):
            """cross-attn e1 loop accumulating abar (mean attn weights)."""
            abar = [None] * 4
            for e1 in range(4):
                sp = scores_all(qa[:, e1, :, :], k4)
                a_l = softmax_tiles(sp)
                feed.pump(fill)
                for e2 in range(4):
                    if e1 == 0:
                        ab = T(smabp, [8, N], BF16, "smab")
                        nc.vector.tensor_copy(out=ab[:], in_=a_l[e2][:])
                        abar[e2] = ab
                    else:
                        nc.vector.tensor_add(out=abar[e2][:], in0=abar[e2][:],
                                             in1=a_l[e2][:])
            return abar

        def cross_fin(mi, abar, v4, dst, feed=None):
            """abar-weighted AV + out proj (wout pre-scaled 0.25)."""
            cacc = T(accp, [128, 8, N], BF16, "big")
            av_accum(abar, v4, cacc[:])
            if feed is not None:
                feed.pump(6)
            drain(gen_projM(wout[mi], HD, HD, lambda kt: cacc[:, kt, :],
                            ev_split(lambda mj: dst[:, mj, :], bout_t[mi],
                                     lambda mj: mj)))

        def gen_gate(g, in_a, in_b, gt):
            return gen_projM(
                wgate[g], HD, 2 * HD,
                lambda kt: in_a[:, kt, :] if kt < 8 else in_b[:, kt - 8, :],
                ev_split(lambda mj: gt[:, mj, :], bgate_t[g],
                         lambda mj: mj, func=AF.Sigmoid))

        def run_selfB(j, enh_dst, sum_dst, feed):
            """LN each position of enh_dst in place; sum_dst = sum_e enh."""
            for e1 in range(4):
                ln_norm(enh_dst[:, :, e1, :], lng_t[j], lnb_t[j],
                        lambda mt, e1=e1: enh_dst[:, mt, e1, :], feed=feed)
                feed.pump(2)
            t2 = T(qkp, [128, 8, N], BF16, "qk")
            nc.vector.tensor_add(out=sum_dst[:], in0=enh_dst[:, :, 0, :],
                                 in1=enh_dst[:, :, 1, :])
            nc.vector.tensor_add(out=t2[:], in0=enh_dst[:, :, 2, :],
                                 in1=enh_dst[:, :, 3, :])
            nc.vector.tensor_add(out=sum_dst[:], in0=sum_dst[:], in1=t2[:])

        def gen_head(c, out):
            """Pass head: x load + expand(0) + kv0 + q0.  Run via a Feed so
            pass c's head can fill pass c-1's tail."""
            bs = slice(c * N, (c + 1) * N)
            xt_b = T(xbp, [128, 8, N], BF16, "xb")
            xs_b = T(xbp, [128, 8, N], BF16, "xb")
            for xd, xb in ((xt_d, xt_b), (xs_d, xs_b)):
                for h in range(4):
                    xf = T(qkp, [128, 2, N], F32, "qk")
                    nc.sync.dma_start(out=xf[:],
                                      in_=xd[:, 2 * h : 2 * h + 2, bs])
                    nc.vector.tensor_copy(out=xb[:, 2 * h : 2 * h + 2, :],
                                          in_=xf[:])
                yield
            xp_t = T(expp, [128, 8, 4, N], BF16, "exp")
            yield from gen_expand(0, xt_b, xp_t)
            k4t = T(qkvp, [128, 4, 8, N], BF16, "qkv")
            v4t = T(qkvp, [128, 4, 8, N], BF16, "qkv")
            yield from gen_kv(0, xp_t, k4t, v4t)
            qa_t = T(acc4p, [128, 4, 8, N], BF16, "acc4")
            yield from gen_q(0, xp_t, qa_t)
            out.update(xt_b=xt_b, xs_b=xs_b, xp_t=xp_t, k4t=k4t, v4t=v4t,
                       qa_t=qa_t)

        heads = [{} for _ in range(NP)]
        head_gens = [gen_head(c, heads[c]) for c in range(NP)]
        drain(head_gens[0])

        for c in range(NP):
            hd = heads[c]
            xt_b, xs_b, xp_t = hd["xt_b"], hd["xs_b"], hd["xp_t"]
            k4t, v4t, qa_t = hd["k4t"], hd["v4t"], hd["qa_t"]

            nc.mark("mid_start")
            t_enh = T(enhp, [128, 8, 4, N], BF16, "enh")
            s_enh = T(enhp, [128, 8, 4, N], BF16, "enh")
            sum_t = T(actp, [128, 8, N], BF16, "big")
            sum_s = T(actp, [128, 8, N], BF16, "big")

            # --- self-t, expand(1) fills the softmax windows
            xp_s = T(expp, [128, 8, 4, N], BF16, "exp")
            f = Feed(gen_expand(1, xs_b, xp_s))
            nc.mark("attn_t")
            attn_loop(qa_t, k4t, v4t, f, fill=3)
            k4s = T(qkvp, [128, 4, 8, N], BF16, "qkv")
            v4s = T(qkvp, [128, 4, 8, N], BF16, "qkv")
            gkv1 = gen_kv(1, xp_s, k4s, v4s)
            f.drain()
            # residual precompute frees xt_b/xs_b before the tail
            cres = (1.0 - res_w) * 0.5
            osum = T(xbp, [128, 8, N], BF16, "xb")
            nc.vector.tensor_add(out=osum[:], in0=xt_b[:], in1=xs_b[:])
            nc.vector.tensor_scalar_mul(osum[:], osum[:], cres)
            # --- kv1 stream, then self-out t
            nc.mark("kv1")
            drain(gkv1)
            nc.mark("self_out0")
            drain(gen_self_out(0, qa_t, xp_t, t_enh))
            # --- q1 with t-LN interleaved
            qa_s = T(acc4p, [128, 4, 8, N], BF16, "acc4")
            f = Feed(gen_q(1, xp_s, qa_s))
            nc.mark("q1_selfB0")
            run_selfB(0, t_enh, sum_t, f)
            f.drain()
            # --- self-s, q2 fills the windows
            qa_c2 = T(acc4p, [128, 4, 8, N], BF16, "acc4")
            f = Feed(gen_q(2, t_enh, qa_c2))
            nc.mark("attn_s")
            attn_loop(qa_s, k4s, v4s, f, fill=2)
            f.drain()
            # --- kv3 chunks cover the last av's adds before self-out s
            k4c3 = T(qkvp, [128, 4, 8, N], BF16, "qkv")
            v4c3 = T(qkvp, [128, 4, 8, N], BF16, "qkv")
            f = Feed(gen_kv(3, t_enh, k4c3, v4c3))
            f.pump(2)
            nc.mark("self_out1")
            drain(gen_self_out(1, qa_s, xp_s, s_enh))
            nc.mark("kv3_selfB1")
            run_selfB(1, s_enh, sum_s, f)
            f.drain()
            # --- q3 stream
            qa_c3 = T(acc4p, [128, 4, 8, N], BF16, "acc4")
            nc.mark("q3")
            drain(gen_q(3, s_enh, qa_c3))
            # --- cross st (mha3): joint/vj/qj fill the windows
            joint = T(actp, [128, 8, N], BF16, "big")
            vj = T(actp, [128, 8, N], BF16, "big")
            qj = T(actp, [128, 8, N], BF16, "big")
            mst = T(meanp, [128, 8, N], BF16, "big")
            mts = T(meanp, [128, 8, N], BF16, "big")
            f = Feed(
                gen_projM(wjoint, HD, 2 * HD,
                          lambda kt: sum_t[:, kt, :] if kt < 8
                          else sum_s[:, kt - 8, :],
                          ev_split(lambda mj: joint[:, mj, :], bjoint_t,
                                   lambda mj: mj)),
                lambda: gen_projM(wqkv[4], HD, HD, lambda kt: joint[:, kt, :],
                                  ev_split(lambda mj: vj[:, mj, :], bqkv_t[4],
                                           lambda mj: 16 + mj),
                                  wcol0=2 * HD),
                lambda: gen_projM(wqkv[4], HD, HD, lambda kt: joint[:, kt, :],
                                  ev_split(lambda mj: qj[:, mj, :], bqkv_t[4],
                                           lambda mj: mj)))
            nc.mark("cross_loop3")
            abar3 = cross_loop(qa_c3, k4c3, f, fill=3)
            f.drain()
            # --- finish cross st; kv2 chunks fill the av window
            k4c2 = T(qkvp, [128, 4, 8, N], BF16, "qkv")
            v4c2 = T(qkvp, [128, 4, 8, N], BF16, "qkv")
            fkv2 = Feed(gen_kv(2, s_enh, k4c2, v4c2))
            nc.mark("crossfin3_kv2")
            cross_fin(3, abar3, v4c3, mst, feed=fkv2)
            fkv2.drain()
            # --- cross ts (mha2): mtj fills the windows
            mtj = T(meanp, [128, 8, N], BF16, "big")
            f = Feed(gen_projM(wout[4], HD, HD, lambda kt: vj[:, kt, :],
                               ev_split(lambda mj: mtj[:, mj, :], bout_t[4],
                                        lambda mj: mj)))
            nc.mark("cross_loop2_mtj")
            abar2 = cross_loop(qa_c2, k4c2, f, fill=1)
            f.drain()
            # --- finish cross ts; kv4t chunks fill the av window
            k4j1 = T(qkvp, [128, 4, 8, N], BF16, "qkv")
            v4j1 = T(qkvp, [128, 4, 8, N], BF16, "qkv")
            fkv4 = Feed(gen_kv(4, t_enh, k4j1, v4j1))
            nc.mark("crossfin2_kv4t")
            cross_fin(2, abar2, v4c2, mts, feed=fkv4)
            fkv4.drain()
            gate_t = T(gatep, [128, 8, N], BF16, "big")
            g0 = Feed(gen_gate(0, mts, mtj, gate_t))
            # --- jx: single-query cross-attn (q = joint row)
            jacc = T(acc4p, [128, 4, 8, N], BF16, "acc4")
            k4j2 = T(qkvp, [128, 4, 8, N], BF16, "qkv")
            v4j2 = T(qkvp, [128, 4, 8, N], BF16, "qkv")
            fkv = Feed(gen_kv(4, s_enh, k4j2, v4j2))
            nxt = Feed()
            if c + 1 < NP:
                nxt.add(head_gens[c + 1])
            nxt.pump(2)   # x loads only: DMA+DVE, no PE instructions
            nc.mark("jx1")
            a_l1 = softmax_tiles(scores_all(qj[:], k4j1))
            fkv.pump(4)
            g0.pump(2)
            av_accum(a_l1, v4j1, jacc[:, 0, :, :])
            fkv.drain()
            nc.mark("jx2")
            a_l2 = softmax_tiles(scores_all(qj[:], k4j2))
            g0.drain()
            # gate_t consumed immediately so gate_s can reuse the pool slot
            f2 = T(accp, [128, 8, N], BF16, "big")
            nc.gpsimd.tensor_tensor(out=f2[:], in0=gate_t[:], in1=mtj[:],
                                    op=ALU.mult)
            nc.gpsimd.tensor_tensor(out=mts[:], in0=gate_t[:], in1=mts[:],
                                    op=ALU.mult)
            gate_s = T(gatep, [128, 8, N], BF16, "big")
            g1 = Feed(gen_gate(1, mst, mtj, gate_s))
            g1.pump(3)
            av_accum(a_l2, v4j2, jacc[:, 1, :, :])
            g1.pump(3)
            mjt = T(meanp, [128, 8, N], BF16, "big")
            mjs = T(meanp, [128, 8, N], BF16, "big")
            def ev_jx(mj, ps):
                for jj, dst in enumerate((mjt, mjs)):
                    nc.scalar.activation(
                        dst[:, mj, :], ps[0][:, jj * 256 : (jj + 1) * 256],
                        AF.Identity, bias=bout_t[4][:, mj : mj + 1])
            nc.mark("evjx")
            drain(gen_projS(wout[4], HD, lambda kt, p: jacc[:, 0:2, kt, :],
                            ev_jx, npair=1))
            g1.drain()
            nc.gpsimd.tensor_tensor(out=mst[:], in0=gate_s[:], in1=mst[:],
                                    op=ALU.mult)
            nc.gpsimd.tensor_tensor(out=mtj[:], in0=gate_s[:], in1=mtj[:],
                                    op=ALU.mult)
            gate_j = T(gatep, [128, 8, N], BF16, "big")
            nc.mark("gate2")
            drain(gen_gate(2, mjt, mjs, gate_j))
            nc.vector.tensor_tensor(out=mjt[:], in0=gate_j[:], in1=mjt[:],
                                    op=ALU.mult)
            nc.vector.tensor_tensor(out=mjs[:], in0=gate_j[:], in1=mjs[:],
                                    op=ALU.mult)
            fs = [mts, mst, f2, mtj, mjt, mjs]

            # --- tail: wo1/wo2/final LN, next pass's head keeps pumping
            nc.mark("tail_wo1")
            nxt.pump(3)
            h1 = T(expp, [128, 8, 4, N], BF16, "exp")
            def ev_h1(mj0, gm, ps):
                for mi in range(gm):
                    m = mj0 + mi
                    nc.scalar.activation(
                        h1[:, m % 8, m // 8, :], ps[mi][:, 0:256],
                        AF.Relu, bias=bo1_t[:, m : m + 1])
            g = gen_projM(wo1, 2 * HD, 6 * HD,
                          lambda kt: fs[kt // 8][:, kt % 8, :], ev_h1)
            i = 0
            for _ in g:
                i += 1
                if i % 4 == 0:
                    nxt.pump(1)
            nc.mark("wo2")
            h2 = T(accp, [128, 8, N], BF16, "big")
            nxt.pump(2)
            g = gen_projM(wo2, HD, 2 * HD,
                          lambda kt: h1[:, kt % 8, kt // 8, :],
                          ev_split(lambda mj: h2[:, mj, :], bo2_t,
                                   lambda mj: mj))
            for _ in g:
                nxt.pump(1)

            # final LN (g,b pre-scaled by res_w) + (1-res_w)/2*(xt+xs)
            nc.mark("final_ln")
            yt = T(qkp, [128, 8, N], BF16, "qk")
            ln_norm(h2[:], lng_t[2], lnb_t[2], lambda mt: yt[:, mt, :],
                    feed=nxt)
            nxt.pump(4)
            nc.vector.tensor_add(out=yt[:], in0=yt[:], in1=osum[:])
            nc.sync.dma_start(out=y_d[:, :, slice(c * N, (c + 1) * N)],
                              in_=yt[:])
            nxt.drain()

    nc.compile()
    return nc


def _sel_const():
    s = np.zeros((8, 8 * 128), np.float32)
    for mt in range(8):
        s[mt, mt * 128 : (mt + 1) * 128] = 1.0
    return s.astype(BF)


def _o32_const():
    o = np.zeros((128, 64), np.float32)
    o[:, 32] = 1.0
    return o.astype(BF)


def _wl(w):
    """torch-style [M_out, K_in] -> pair-blocked [128, M/256, K/128, 256] bf16
    (one 2-mtile all-kt block contiguous per partition)."""
    a = np.asarray(w, np.float32).T          # [K, M]
    K, M = a.shape
    a = a.reshape(K // 128, 128, M // 256, 256).transpose(1, 2, 0, 3)
    return np.ascontiguousarray(a).astype(BF)


def _prep_inputs(i):
    res_w = float(np.asarray(i["res_w"]).reshape(-1)[0])
    sc = 1.0 / math.sqrt(128.0)

    shared = {
        "wexp0": _wl(i["exp_t_w"]), "wexp1": _wl(i["exp_s_w"]),
        "bexp0": _bias_cols(np.asarray(i["exp_t_b"]) + np.asarray(i["pos_enc"]).reshape(-1)),
        "bexp1": _bias_cols(np.asarray(i["exp_s_b"]) + np.asarray(i["pos_enc"]).reshape(-1)),
        "wjoint": _wl(np.asarray(i["joint_w"], np.float32) * 0.25),
        "bjoint": _bias_cols(i["joint_b"]),
        "wo1": _wl(i["out1_w"]), "bo1": _bias_cols(i["out1_b"]),
        "wo2": _wl(i["out2_w"]), "bo2": _bias_cols(i["out2_b"]),
        "sel_c": _sel_const(), "o32_c": _o32_const(),
    }
    for g in range(3):
        shared[f"wgate{g}"] = _wl(i["gate_w"][g])
        shared[f"bgate{g}"] = _bias_cols(i["gate_b"][g])
    for m in range(5):
        w = np.asarray(i["mha_in_w"][m], np.float32).copy()
        b = np.asarray(i["mha_in_b"][m], np.float32).copy()
        w[:HD] *= sc
        b[:HD] *= sc
        shared[f"wqkv{m}"] = _wl(w)
        shared[f"bqkv{m}"] = _bias_cols(b)
        wo = np.asarray(i["mha_out_w"][m], np.float32)
        if m in (2, 3):
            wo = wo * 0.25      # fold mean over the 4 query positions
        shared[f"wout{m}"] = _wl(wo)
        shared[f"bout{m}"] = _bias_cols(i["mha_out_b"][m])
    for ln in range(3):
        g = np.asarray(i["ln_g"][ln], np.float32)
        b = np.asarray(i["ln_b"][ln], np.float32)
        if ln == 2:
            g = g * res_w
            b = b * res_w
        shared[f"lng{ln}"] = _bias_cols(g)
        shared[f"lnb{ln}"] = _bias_cols(b)

    def shard_x(x, c):
        xc = np.asarray(x, np.float32)[c * BC : (c + 1) * BC, 0, :]  # [512,1024]
        return np.ascontiguousarray(xc.T.reshape(8, 128, BC).transpose(1, 0, 2))

    in_maps = []
    for c in range(NCORES):
        m = dict(shared)
        m["xt"] = shard_x(i["temporal_features"], c)
        m["xs"] = shard_x(i["spatial_features"], c)
        in_maps.append(m)
    return res_w, in_maps


def kernel(**inputs):
    res_w, in_maps = _prep_inputs(inputs)
    nc = build(res_w)
    res = bass_utils.run_bass_kernel_spmd(nc, in_maps, core_ids=list(range(NCORES)))
    outs = []
    for c in range(NCORES):
        y = res.results[c]["y"]                                   # [128,8,512]
        outs.append(np.asarray(y).astype(np.float32)
                    .transpose(1, 0, 2).reshape(HD, BC).T)
    return np.concatenate(outs, 0)[:, None, :].astype(np.float32)


# revision 25
# speedup vs baseline: 1.0559x; 1.0174x over previous
"""Trainium2 Bass kernel for nn_MISA (dense_transformer, data-parallel over 8 cores).

Layout: feature-major activations [feat_part=128, mtile, batch_cols] per core.
Batch 4096 -> 512 per core -> two passes of 256 columns.
All matmuls bf16 (fp32 PSUM accumulation); LN/softmax internals fp32.

v3 (PE-gap removal): the PE is the bottleneck engine; v2 lost ~400us to PE
idle gaps waiting on DVE softmax/LN chains plus ~300us of HAM cold-throttle
restarts those gaps caused.  v3 keeps the PE warm:
- every projection is a generator yielding at m-tile boundaries; a Feed
  object interleaves pending projection chunks into every DVE-bound window
  (softmax, LayerNorm, gate chains) as PE filler.
- attention loops are software-pipelined: scores(e1) / softmax(e1) /
  filler / av(e1-1), so av's dependency on softmax is always satisfied by
  the time the PE reaches it.
- av accumulation adds run on GpSimd (otherwise idle), LN's per-tile
  affine (g,b) runs on the Scalar engine, self-attn residual adds moved
  from PE identity-matmuls to DVE.
- pass c+1's head (x load, expand, kv0, q0) is pumped as PE filler during
  pass c's tail (wo1/wo2/final LN), removing the inter-pass PE gap.

Structural simplifications (exact, not approximations):
- attention with all-equal keys/values (q/k/v = broadcast joint row) is the
  identity on v: cross_tj == cross_sj == out_proj4(v_proj4(joint)).
- mean over query positions commutes with out_proj and with A@V, so the six
  cross outputs never materialize per-query outputs (abar-weighted V only).
- all-equal queries (j as q): single query row, output equals its mean.
"""
import sys, math
from contextlib import ExitStack
sys.path.insert(0, "/opt/trn_rl_repo")

import numpy as np
import ml_dtypes

import concourse.bass as bass
import concourse.mybir as mybir
from concourse import bacc
import concourse.tile as tile
from concourse import bass_utils

F32 = mybir.dt.float32
BF16 = mybir.dt.bfloat16
AF = mybir.ActivationFunctionType
ALU = mybir.AluOpType
BF = ml_dtypes.bfloat16

H = 8
E = 4
HD = 1024
B = 4096
NCORES = 8
BC = B // NCORES          # 512 batch per core
NP = 2                    # passes per core
N = BC // NP              # 256 batch cols per pass
EPS = 1e-5


def _bias_cols(b):
    # [M] -> [128, M//128]: column m = per-partition bias of m-tile m
    return np.ascontiguousarray(np.asarray(b, np.float32).reshape(-1, 128).T)


class Feed:
    """Queue of projection generators (or factories) pumped as PE filler.
    Factories instantiate lazily when reached, so their eager first weight
    DMA overlaps the tail of the preceding stream."""
    def __init__(self, *gens):
        self.q = list(gens)

    def add(self, g):
        self.q.append(g)

    def pump(self, n=1):
        while n > 0 and self.q:
            g = self.q[0]
            if not hasattr(g, "__next__"):
                g = self.q[0] = g()
            try:
                next(g)
                n -= 1
            except StopIteration:
                self.q.pop(0)

    def drain(self):
        while self.q:
            self.pump(1)


def drain(g):
    for _ in g:
        pass


def build(res_w: float):
    nc = bacc.Bacc("TRN2", target_bir_lowering=False, debug=False)

    def din(name, shape, dt):
        return nc.dram_tensor(name, list(shape), dt, kind="ExternalInput").ap()

    xt_d = din("xt", (128, 8, BC), F32)
    xs_d = din("xs", (128, 8, BC), F32)
    # weights pair-blocked: [128, M/256, K/128, 256] — one (2-mtile, all-kt)
    # block is contiguous per partition, so block DMAs run at full rate
    wexp = [din(f"wexp{j}", (128, 16, 8, 256), BF16) for j in range(2)]
    bexp = [din(f"bexp{j}", (128, 32), F32) for j in range(2)]
    wqkv = [din(f"wqkv{i}", (128, 12, 8, 256), BF16) for i in range(5)]
    bqkv = [din(f"bqkv{i}", (128, 24), F32) for i in range(5)]
    wout = [din(f"wout{i}", (128, 4, 8, 256), BF16) for i in range(5)]
    bout = [din(f"bout{i}", (128, 8), F32) for i in range(5)]
    wjoint = din("wjoint", (128, 4, 16, 256), BF16)
    bjoint = din("bjoint", (128, 8), F32)
    wgate = [din(f"wgate{g}", (128, 4, 16, 256), BF16) for g in range(3)]
    bgate = [din(f"bgate{g}", (128, 8), F32) for g in range(3)]
    wo1 = din("wo1", (128, 8, 48, 256), BF16)
    bo1 = din("bo1", (128, 16), F32)
    wo2 = din("wo2", (128, 4, 16, 256), BF16)
    bo2 = din("bo2", (128, 8), F32)
    lng = [din(f"lng{i}", (128, 8), F32) for i in range(3)]
    lnb = [din(f"lnb{i}", (128, 8), F32) for i in range(3)]
    sel_d = din("sel_c", (8, 8 * 128), BF16)
    o32_d = din("o32_c", (128, 64), BF16)
    y_d = nc.dram_tensor("y", [128, 8, BC], BF16, kind="ExternalOutput").ap()

    with tile.TileContext(nc) as tc, ExitStack() as ctx:
        P = lambda **kw: ctx.enter_context(tc.tile_pool(**kw))
        cst = P(name="cst", bufs=1)
        wgp = P(name="wgp", bufs=3)                 # [128,8,256] weight blocks
        mmp = P(name="mmp", bufs=4, space="PSUM")   # 4 x [128,512] banks
        scp = P(name="scp", bufs=1, space="PSUM")   # [8,4,N] = 2 banks
        brp = P(name="brp", bufs=2, space="PSUM")   # 2 x [128,N] banks
        expp = P(name="expp", bufs=2)   # xp_t, xp_s; also h1, next xp_t
        enhp = P(name="enhp", bufs=2)   # t_enh, s_enh [128,8,4,N]
        qkvp = P(name="qkvp", bufs=2)   # k4, v4 [128,4,8,N]
        acc4p = P(name="acc4p", bufs=2)  # qa tiles + jacc [128,4,8,N]
        bigp = P(name="bigp", bufs=7)   # all long-lived [128,8,N] bf16
        actp = meanp = accp = gatep = bigp
        qkp = P(name="qkp", bufs=2)     # qk products, av curs, ln scratch
        xbp = P(name="xbp", bufs=3)     # xt_b, xs_b, osum
        smp = P(name="smp", bufs=1)     # softmax exp [8,4,N] bf16
        smdp = P(name="smdp", bufs=2)   # softmax denom [8,N] f32
        smbp = P(name="smbp", bufs=3)   # a_l bf16 [8,N]
        smrp = P(name="smrp", bufs=3)   # LN scalars [1,N] f32
        smabp = P(name="smabp", bufs=4)  # abar f32 [8,N]

        nc._phase_marks = []
        def mark(name):
            nc._phase_marks.append((name, nc.next_id()))
        nc.mark = mark

        _tc = [0]
        def T(pool, shape, dtype, tag):
            _tc[0] += 1
            return pool.tile(shape, dtype, tag=tag, name=f"{tag}_{_tc[0]}")

        ones_b = T(cst, [128, 1], BF16, "ones_b")
        nc.any.memset(ones_b[:], 1.0)
        onerow_f = T(cst, [1, 128], F32, "onerow_f")
        nc.any.memset(onerow_f[:], 1.0)
        sel = T(cst, [8, 8 * 128], BF16, "sel")
        nc.sync.dma_start(out=sel[:], in_=sel_d)
        o32 = T(cst, [128, 64], BF16, "o32")   # all-ones column at col 32
        nc.sync.dma_start(out=o32[:], in_=o32_d)
        eps_t = T(cst, [1, 1], F32, "eps_t")
        nc.any.memset(eps_t[:], EPS)

        def ctile(name, ap):
            t = cst.tile(list(ap.shape), ap.dtype, tag=name)
            nc.sync.dma_start(out=t[:], in_=ap)
            return t

        bexp_t = [ctile(f"bexp{j}", bexp[j]) for j in range(2)]
        bqkv_t = [ctile(f"bqkv{i}", bqkv[i]) for i in range(5)]
        bout_t = [ctile(f"bout{i}", bout[i]) for i in range(5)]
        bjoint_t = ctile("bjoint", bjoint)
        bgate_t = [ctile(f"bgate{g}", bgate[g]) for g in range(3)]
        bo1_t = ctile("bo1", bo1)
        bo2_t = ctile("bo2", bo2)
        lng_t = [ctile(f"lng{i}", lng[i]) for i in range(3)]
        lnb_t = [ctile(f"lnb{i}", lnb[i]) for i in range(3)]

        def gen_projS(w_d, M, src_pair, evict2, wcol0=0, npair=2):
            """Shared-weight projection, K=1024. src_pair(kt, p) -> [128,2,N]
            moving pair; two pairs (4 sources) per stationary load.
            evict2(mj, psums): psums[p] = [128,512] = pair p's two outputs.
            Yields once per mj (16 matmuls).  The first weight DMA issues
            eagerly at call time; later blocks prefetch one group ahead."""
            nmt = M // 128
            blocks = list(range(0, nmt, 2))
            wts = {}
            def load(mj0, split=False):
                wt = T(wgp, [128, 8, 256], BF16, "wg")
                blk = w_d[:, (wcol0 + mj0 * 128) // 256, :, :]
                if split:
                    nc.sync.dma_start(out=wt[:, 0:4, :], in_=blk[:, 0:4, :])
                    nc.sync.dma_start(out=wt[:, 4:8, :], in_=blk[:, 4:8, :])
                else:
                    nc.sync.dma_start(out=wt[:], in_=blk)
                wts[mj0] = wt
            load(blocks[0], split=True)
            def g():
                for bi, mj0 in enumerate(blocks):
                    gm = min(2, nmt - mj0)
                    wt = wts.pop(mj0)
                    for mj in range(mj0, mj0 + gm):
                        ps = [T(mmp, [128, 512], F32, "mm")
                              for _ in range(npair)]
                        for kt in range(8):
                            w_sl = wt[:, kt,
                                      (mj - mj0) * 128 : (mj - mj0 + 1) * 128]
                            for p in range(npair):
                                nc.tensor.matmul(ps[p][:], w_sl,
                                                 src_pair(kt, p),
                                                 start=(kt == 0),
                                                 stop=(kt == 7))
                        evict2(mj, ps)
                        if mj == mj0 and bi + 1 < len(blocks):
                            load(blocks[bi + 1])
                        yield
            return g()

        def gen_projM(w_d, M, K, src, evict2m, wcol0=0):
            """Single-source projection. One PSUM bank per m-tile (interleaved
            accumulation groups must not share a bank: start=True clears the
            has-written bits bank-wide). evict2m(mj0, gm, ps_list).
            Yields once per 8-kt chunk (16 matmuls at gm=2).  First weight DMA
            issues eagerly at call time; later chunks prefetch one ahead."""
            nmt, nkt = M // 128, K // 128
            steps = [(mj0, kc0) for mj0 in range(0, nmt, 2)
                     for kc0 in range(0, nkt, 8)]
            wts = {}
            def load(i, split=False):
                mj0, kc0 = steps[i]
                kc = min(8, nkt - kc0)
                wt = T(wgp, [128, 8, 256], BF16, "wg")
                blk = w_d[:, (wcol0 + mj0 * 128) // 256, kc0 : kc0 + kc, :]
                if split and kc == 8:
                    nc.sync.dma_start(out=wt[:, 0:4, :], in_=blk[:, 0:4, :])
                    nc.sync.dma_start(out=wt[:, 4:8, :], in_=blk[:, 4:8, :])
                else:
                    nc.sync.dma_start(out=wt[:, :kc, :], in_=blk)
                wts[i] = wt
            load(0, split=True)
            def g():
                ps = None
                for i, (mj0, kc0) in enumerate(steps):
                    gm = min(2, nmt - mj0)
                    if kc0 == 0:
                        ps = [T(mmp, [128, 512], F32, "mm") for _ in range(gm)]
                    kc = min(8, nkt - kc0)
                    wt = wts.pop(i)
                    if i + 1 < len(steps):
                        load(i + 1)
                    for kt in range(kc0, kc0 + kc):
                        s = src(kt)
                        for mi in range(gm):
                            nc.tensor.matmul(
                                ps[mi][:, 0:256],
                                wt[:, kt - kc0, mi * 128 : (mi + 1) * 128], s,
                                start=(kt == 0), stop=(kt == nkt - 1))
                    if kc0 + 8 >= nkt:
                        evict2m(mj0, gm, ps)
                    yield
            return g()

        def ev_split(dsts_of_mj, btile, bcol_of_mj, func=AF.Identity):
            """projM eviction: per-mtile ACT evicts [128,256] with bias."""
            def _ev(mj0, gm, ps):
                for mi in range(gm):
                    nc.scalar.activation(
                        dsts_of_mj(mj0 + mi), ps[mi][:, 0:256],
                        func, bias=btile[:, bcol_of_mj(mj0 + mi)
                                         : bcol_of_mj(mj0 + mi) + 1])
            return _ev

        def scores_all(q_sl, k4):
            """psum [8,4,N]: row h of col-block e2 = q[h].k[e2,h] (q pre-scaled).
            q_sl [128,8,N] contiguous; k4 [128,4,8,N] e-major."""
            sp = T(scp, [8, 4, N], F32, "sc")
            for e2 in range(4):
                p = T(qkp, [128, 8, N], BF16, "qk")
                nc.vector.tensor_tensor(
                    out=p[:], in0=q_sl, in1=k4[:, e2, :, :], op=ALU.mult)
                for kt in range(8):
                    nc.tensor.matmul(sp[:, e2, :], o32[:, 32 - kt : 40 - kt],
                                     p[:, kt, :], start=(kt == 0), stop=(kt == 7))
            return sp

        def softmax_tiles(sp):
            """sp [8,4,N] psum scores -> 4 bf16 [8,N] attention-weight tiles."""
            et = T(smp, [8, 4, N], BF16, "sm")
            nc.scalar.activation(et[:], sp[:], AF.Exp)
            d = T(smdp, [8, N], F32, "smd")
            nc.vector.tensor_add(out=d[:], in0=et[:, 0, :], in1=et[:, 1, :])
            for e2 in (2, 3):
                nc.vector.tensor_add(out=d[:], in0=d[:], in1=et[:, e2, :])
            r = T(smdp, [8, N], F32, "smd")
            nc.vector.reciprocal_approx_fast(out=r[:], in_=d[:])
            outs = []
            for e2 in range(4):
                a = T(smbp, [8, N], BF16, "smb")
                nc.vector.tensor_tensor(out=a[:], in0=et[:, e2, :], in1=r[:],
                                        op=ALU.mult)
                outs.append(a)
            return outs

        def av_accum(a_list, v4, dst_sl):
            """dst_sl [128,8,N] contiguous = sum_e2 bcast(a_list[e2]) * V[e2].
            v4 [128,4,8,N] e-major.  All DVE; one scratch cur reused so the
            adds interleave with the next e2's mults."""
            for e2 in range(4):
                cur = dst_sl if e2 == 0 else T(qkp, [128, 8, N], BF16, "qk")
                for mt in range(0, 8, 2):
                    bp = T(brp, [128, 2, N], F32, "br")
                    for q in range(2):
                        nc.tensor.matmul(
                            bp[:, q, :],
                            sel[:, (mt + q) * 128 : (mt + q + 1) * 128],
                            a_list[e2][:], start=True, stop=True)
                    nc.vector.tensor_tensor(
                        out=cur[:, mt : mt + 2, :], in0=bp[:],
                        in1=v4[:, e2, mt : mt + 2, :], op=ALU.mult)
                if e2 > 0:
                    nc.vector.tensor_add(out=dst_sl, in0=dst_sl, in1=cur[:])

        def ln_norm(x_sl, g_t, b_t, dst_of_mt, feed=None):
            """LayerNorm over the 1024 feats of x_sl [128,8,N] (bf16, in-place
            scratch); writes normalized*g+b to dst_of_mt(mt).  Per-mt chain:
            2 DVE ops (all-bf16 SBUF, 2x mode) + 1 Scalar affine."""
            sq = T(qkp, [128, 8, N], BF16, "qk")
            nc.vector.tensor_tensor(out=sq[:], in0=x_sl, in1=x_sl, op=ALU.mult)
            st_s = T(brp, [1, N], F32, "br")
            for kt in range(8):
                nc.tensor.matmul(st_s[:], ones_b[:], x_sl[:, kt, :],
                                 start=(kt == 0), stop=(kt == 7))
            st_q = T(brp, [1, N], F32, "br")
            for kt in range(8):
                nc.tensor.matmul(st_q[:], ones_b[:], sq[:, kt, :],
                                 start=(kt == 0), stop=(kt == 7))
            mean = T(smrp, [1, N], F32, "smr")
            nc.vector.tensor_scalar_mul(mean[:], st_s[:], 1.0 / HD)
            mb = T(brp, [128, N], F32, "br")
            nc.tensor.matmul(mb[:], onerow_f[:], mean[:], start=True, stop=True)
            msq = T(smrp, [1, N], F32, "smr")
            nc.vector.tensor_scalar_mul(msq[:], st_q[:], 1.0 / HD)
            var = T(smrp, [1, N], F32, "smr")
            nc.vector.tensor_tensor(out=var[:], in0=mean[:], in1=mean[:],
                                    op=ALU.mult)
            nc.vector.tensor_tensor(out=var[:], in0=msq[:], in1=var[:],
                                    op=ALU.subtract)
            std = T(smrp, [1, N], F32, "smr")
            nc.scalar.activation(std[:], var[:], AF.Sqrt, bias=eps_t[:])
            rstd = T(smrp, [1, N], F32, "smr")
            nc.vector.reciprocal_approx_fast(out=rstd[:], in_=std[:])
            rb = T(brp, [128, N], F32, "br")
            nc.tensor.matmul(rb[:], onerow_f[:], rstd[:], start=True, stop=True)
            # bf16 SBUF copies of the broadcasts
            mbb = T(qkp, [128, 2, N], BF16, "qk")
            nc.scalar.activation(mbb[:, 0, :], mb[:], AF.Identity, bias=0.0)
            nc.scalar.activation(mbb[:, 1, :], rb[:], AF.Identity, bias=0.0)
            if feed is not None:
                feed.pump(1)
            for mt in range(8):
                nc.vector.tensor_tensor(out=x_sl[:, mt, :], in0=x_sl[:, mt, :],
                                        in1=mbb[:, 0, :], op=ALU.subtract)
                nc.vector.tensor_tensor(out=x_sl[:, mt, :], in0=x_sl[:, mt, :],
                                        in1=mbb[:, 1, :], op=ALU.mult)
                nc.scalar.activation(
                    dst_of_mt(mt), x_sl[:, mt, :], AF.Identity,
                    bias=b_t[:, mt : mt + 1], scale=g_t[:, mt : mt + 1])

        def gen_kv(mi, src4, k4, v4):
            """K/V projection of mha mi from src4 [128,8(kt),4(e),N] ->
            k4, v4 [128,4(e),8(mt),N] e-major."""
            def ev(mj, ps):
                dst = k4 if mj < 8 else v4
                bcol = 8 + mj      # k tiles: cols 8..15, v tiles: 16..23
                for p in range(2):
                    for q in range(2):
                        nc.scalar.activation(
                            dst[:, 2 * p + q, mj % 8, :],
                            ps[p][:, q * 256 : (q + 1) * 256],
                            AF.Identity,
                            bias=bqkv_t[mi][:, bcol : bcol + 1])
            return gen_projS(
                wqkv[mi], 2 * HD,
                lambda kt, p: src4[:, kt, 2 * p : 2 * p + 2, :], ev, wcol0=HD)

        def gen_q(mi, src4, qa):
            """q projection into a merged q/acc tile: scores consume the e1
            slice, then av_accum overwrites it with the AV result in place."""
            def ev(mj, ps):
                for p in range(2):
                    for q in range(2):
                        nc.scalar.activation(
                            qa[:, 2 * p + q, mj, :],
                            ps[p][:, q * 256 : (q + 1) * 256],
                            AF.Identity, bias=bqkv_t[mi][:, mj : mj + 1])
            return gen_projS(
                wqkv[mi], HD,
                lambda kt, p: src4[:, kt, 2 * p : 2 * p + 2, :], ev)

        def gen_expand(j, x_b, xp):
            # expand: m-tile m = e*8+mj -> xp[:, mj, e, :]
            def ev_exp(mj0, gm, ps):
                for mi in range(gm):
                    m = mj0 + mi
                    nc.scalar.activation(
                        xp[:, m % 8, m // 8, :], ps[mi][:, 0:256],
                        AF.Identity, bias=bexp_t[j][:, m : m + 1])
            return gen_projM(wexp[j], E * HD, HD,
                             lambda kt: x_b[:, kt, :], ev_exp)

        def gen_self_out(j, qa, xp, enh_dst):
            """out proj -> enh_dst pre-LN; residual added on DVE."""
            def ev_out(mj, ps):
                for p in range(2):
                    nc.scalar.activation(
                        enh_dst[:, mj, 2 * p : 2 * p + 2, :], ps[p][:],
                        AF.Identity, bias=bout_t[j][:, mj : mj + 1])
                nc.vector.tensor_tensor(
                    out=enh_dst[:, mj, :, :], in0=enh_dst[:, mj, :, :],
                    in1=xp[:, mj, :, :], op=ALU.add)
            return gen_projS(
                wout[j], HD,
                lambda kt, p: qa[:, 2 * p : 2 * p + 2, kt, :], ev_out)

        def attn_loop(qa, k4, v4, feed, fill=2):
            """self-attn e1 loop, software-pipelined; av result replaces q in
            qa in place one iteration late."""
            prev = None
            for e1 in range(4):
                sp = scores_all(qa[:, e1, :, :], k4)
                a_l = softmax_tiles(sp)
                feed.pump(fill)
                if prev is not None:
                    av_accum(prev, v4, qa[:, e1 - 1, :, :])
                prev = a_l
            feed.pump(fill)
            av_accum(prev, v4, qa[:, 3, :, :])

        def cross_loop(qa, k4, feed, fill=2):
            """cross-attn e1 loop accumulating abar (mean attn weights)."""
            abar = [None] * 4
            for e1 in range(4):
                sp = scores_all(qa[:, e1, :, :], k4)
                a_l = softmax_tiles(sp)
                feed.pump(fill)
                for e2 in range(4):
                    if e1 == 0:
                        ab = T(smabp, [8, N], BF16, "smab")
                        nc.vector.tensor_copy(out=ab[:], in_=a_l[e2][:])
                        abar[e2] = ab
                    else:
                        nc.vector.tensor_add(out=abar[e2][:], in0=abar[e2][:],
                                             in1=a_l[e2][:])
            return abar

        def cross_fin(mi, abar, v4, dst, feed=None):
            """abar-weighted AV + out proj (wout pre-scaled 0.25)."""
            cacc = T(accp, [128, 8, N], BF16, "big")
            av_accum(abar, v4, cacc[:])
            if feed is not None:
                feed.pump(6)
            drain(gen_projM(wout[mi], HD, HD, lambda kt: cacc[:, kt, :],
                            ev_split(lambda mj: dst[:, mj, :], bout_t[mi],
                                     lambda mj: mj)))

        def gen_gate(g, in_a, in_b, gt):
            return gen_projM(
                wgate[g], HD, 2 * HD,
                lambda kt: in_a[:, kt, :] if kt < 8 else in_b[:, kt - 8, :],
                ev_split(lambda mj: gt[:, mj, :], bgate_t[g],
                         lambda mj: mj, func=AF.Sigmoid))

        def run_selfB(j, enh_dst, sum_dst, feed):
            """LN each position of enh_dst in place; sum_dst = sum_e enh."""
            for e1 in range(4):
                ln_norm(enh_dst[:, :, e1, :], lng_t[j], lnb_t[j],
                        lambda mt, e1=e1: enh_dst[:, mt, e1, :], feed=feed)
                feed.pump(2)
            t2 = T(qkp, [128, 8, N], BF16, "qk")
            nc.vector.tensor_add(out=sum_dst[:], in0=enh_dst[:, :, 0, :],
                                 in1=enh_dst[:, :, 1, :])
            nc.vector.tensor_add(out=t2[:], in0=enh_dst[:, :, 2, :],
                                 in1=enh_dst[:, :, 3, :])
            nc.vector.tensor_add(out=sum_dst[:], in0=sum_dst[:], in1=t2[:])

        def gen_head(c, out):
            """Pass head: x load + expand(0) + kv0 + q0.  Run via a Feed so
            pass c's head can fill pass c-1's tail."""
            bs = slice(c * N, (c + 1) * N)
            xt_b = T(xbp, [128, 8, N], BF16, "xb")
            xs_b = T(xbp, [128, 8, N], BF16, "xb")
            for xd, xb in ((xt_d, xt_b), (xs_d, xs_b)):
                for h in range(4):
                    xf = T(qkp, [128, 2, N], F32, "qk")
                    nc.sync.dma_start(out=xf[:],
                                      in_=xd[:, 2 * h : 2 * h + 2, bs])
                    nc.vector.tensor_copy(out=xb[:, 2 * h : 2 * h + 2, :],
                                          in_=xf[:])
                yield
            xp_t = T(expp, [128, 8, 4, N], BF16, "exp")
            yield from gen_expand(0, xt_b, xp_t)
            k4t = T(qkvp, [128, 4, 8, N], BF16, "qkv")
            v4t = T(qkvp, [128, 4, 8, N], BF16, "qkv")
            yield from gen_kv(0, xp_t, k4t, v4t)
            qa_t = T(acc4p, [128, 4, 8, N], BF16, "acc4")
            yield from gen_q(0, xp_t, qa_t)
            out.update(xt_b=xt_b, xs_b=xs_b, xp_t=xp_t, k4t=k4t, v4t=v4t,
                       qa_t=qa_t)

        heads = [{} for _ in range(NP)]
        head_gens = [gen_head(c, heads[c]) for c in range(NP)]
        drain(head_gens[0])

        for c in range(NP):
            hd = heads[c]
            xt_b, xs_b, xp_t = hd["xt_b"], hd["xs_b"], hd["xp_t"]
            k4t, v4t, qa_t = hd["k4t"], hd["v4t"], hd["qa_t"]

            nc.mark("mid_start")
            t_enh = T(enhp, [128, 8, 4, N], BF16, "enh")
            s_enh = T(enhp, [128, 8, 4, N], BF16, "enh")
            sum_t = T(actp, [128, 8, N], BF16, "big")
            sum_s = T(actp, [128, 8, N], BF16, "big")

            # --- self-t, expand(1) fills the softmax windows
            xp_s = T(expp, [128, 8, 4, N], BF16, "exp")
            f = Feed(gen_expand(1, xs_b, xp_s))
            nc.mark("attn_t")
            attn_loop(qa_t, k4t, v4t, f, fill=3)
            k4s = T(qkvp, [128, 4, 8, N], BF16, "qkv")
            v4s = T(qkvp, [128, 4, 8, N], BF16, "qkv")
            gkv1 = gen_kv(1, xp_s, k4s, v4s)
            f.drain()
            # residual precompute frees xt_b/xs_b before the tail
            cres = (1.0 - res_w) * 0.5
            osum = T(xbp, [128, 8, N], BF16, "xb")
            nc.vector.tensor_add(out=osum[:], in0=xt_b[:], in1=xs_b[:])
            nc.vector.tensor_scalar_mul(osum[:], osum[:], cres)
            # --- kv1 stream, then self-out t
            nc.mark("kv1")
            drain(gkv1)
            nc.mark("self_out0")
            drain(gen_self_out(0, qa_t, xp_t, t_enh))
            # --- q1 with t-LN interleaved
            qa_s = T(acc4p, [128, 4, 8, N], BF16, "acc4")
            f = Feed(gen_q(1, xp_s, qa_s))
            nc.mark("q1_selfB0")
            run_selfB(0, t_enh, sum_t, f)
            f.drain()
            # --- self-s, q2 fills the windows
            qa_c2 = T(acc4p, [128, 4, 8, N], BF16, "acc4")
            f = Feed(gen_q(2, t_enh, qa_c2))
            nc.mark("attn_s")
            attn_loop(qa_s, k4s, v4s, f, fill=2)
            f.drain()
            # --- kv3 chunks cover the last av's adds before self-out s
            k4c3 = T(qkvp, [128, 4, 8, N], BF16, "qkv")
            v4c3 = T(qkvp, [128, 4, 8, N], BF16, "qkv")
            f = Feed(gen_kv(3, t_enh, k4c3, v4c3))
            f.pump(2)
            nc.mark("self_out1")
            drain(gen_self_out(1, qa_s, xp_s, s_enh))
            nc.mark("kv3_selfB1")
            run_selfB(1, s_enh, sum_s, f)
            f.drain()
            # --- q3 stream
            qa_c3 = T(acc4p, [128, 4, 8, N], BF16, "acc4")
            nc.mark("q3")
            drain(gen_q(3, s_enh, qa_c3))
            # --- cross st (mha3): joint/vj/qj fill the windows
            joint = T(actp, [128, 8, N], BF16, "big")
            vj = T(actp, [128, 8, N], BF16, "big")
            qj = T(actp, [128, 8, N], BF16, "big")
            mst = T(meanp, [128, 8, N], BF16, "big")
            mts = T(meanp, [128, 8, N], BF16, "big")
            f = Feed(
                gen_projM(wjoint, HD, 2 * HD,
                          lambda kt: sum_t[:, kt, :] if kt < 8
                          else sum_s[:, kt - 8, :],
                          ev_split(lambda mj: joint[:, mj, :], bjoint_t,
                                   lambda mj: mj)),
                lambda: gen_projM(wqkv[4], HD, HD, lambda kt: joint[:, kt, :],
                                  ev_split(lambda mj: vj[:, mj, :], bqkv_t[4],
                                           lambda mj: 16 + mj),
                                  wcol0=2 * HD),
                lambda: gen_projM(wqkv[4], HD, HD, lambda kt: joint[:, kt, :],
                                  ev_split(lambda mj: qj[:, mj, :], bqkv_t[4],
                                           lambda mj: mj)))
            nc.mark("cross_loop3")
            abar3 = cross_loop(qa_c3, k4c3, f, fill=3)
            f.drain()
            # --- finish cross st; kv2 chunks fill the av window
            k4c2 = T(qkvp, [128, 4, 8, N], BF16, "qkv")
            v4c2 = T(qkvp, [128, 4, 8, N], BF16, "qkv")
            fkv2 = Feed(gen_kv(2, s_enh, k4c2, v4c2))
            nc.mark("crossfin3_kv2")
            cross_fin(3, abar3, v4c3, mst, feed=fkv2)
            fkv2.drain()
            # --- cross ts (mha2): mtj fills the windows
            mtj = T(meanp, [128, 8, N], BF16, "big")
            f = Feed(gen_projM(wout[4], HD, HD, lambda kt: vj[:, kt, :],
                               ev_split(lambda mj: mtj[:, mj, :], bout_t[4],
                                        lambda mj: mj)))
            nc.mark("cross_loop2_mtj")
            abar2 = cross_loop(qa_c2, k4c2, f, fill=1)
            f.drain()
            # --- finish cross ts; kv4t chunks fill the av window
            k4j1 = T(qkvp, [128, 4, 8, N], BF16, "qkv")
            v4j1 = T(qkvp, [128, 4, 8, N], BF16, "qkv")
            fkv4 = Feed(gen_kv(4, t_enh, k4j1, v4j1))
            nc.mark("crossfin2_kv4t")
            cross_fin(2, abar2, v4c2, mts, feed=fkv4)
            fkv4.drain()
            gate_t = T(gatep, [128, 8, N], BF16, "big")
            g0 = Feed(gen_gate(0, mts, mtj, gate_t))
            # --- jx: single-query cross-attn (q = joint row)
            jacc = T(acc4p, [128, 4, 8, N], BF16, "acc4")
            k4j2 = T(qkvp, [128, 4, 8, N], BF16, "qkv")
            v4j2 = T(qkvp, [128, 4, 8, N], BF16, "qkv")
            fkv = Feed(gen_kv(4, s_enh, k4j2, v4j2))
            nc.mark("jx1")
            a_l1 = softmax_tiles(scores_all(qj[:], k4j1))
            fkv.pump(4)
            g0.pump(2)
            av_accum(a_l1, v4j1, jacc[:, 0, :, :])
            fkv.drain()
            nc.mark("jx2")
            a_l2 = softmax_tiles(scores_all(qj[:], k4j2))
            g0.drain()
            # gate_t consumed immediately so gate_s can reuse the pool slot
            f2 = T(accp, [128, 8, N], BF16, "big")
            nc.gpsimd.tensor_tensor(out=f2[:], in0=gate_t[:], in1=mtj[:],
                                    op=ALU.mult)
            nc.gpsimd.tensor_tensor(out=mts[:], in0=gate_t[:], in1=mts[:],
                                    op=ALU.mult)
            gate_s = T(gatep, [128, 8, N], BF16, "big")
            g1 = Feed(gen_gate(1, mst, mtj, gate_s))
            g1.pump(3)
            av_accum(a_l2, v4j2, jacc[:, 1, :, :])
            g1.pump(3)
            mjt = T(meanp, [128, 8, N], BF16, "big")
            mjs = T(meanp, [128, 8, N], BF16, "big")
            def ev_jx(mj, ps):
                for jj, dst in enumerate((mjt, mjs)):
                    nc.scalar.activation(
                        dst[:, mj, :], ps[0][:, jj * 256 : (jj + 1) * 256],
                        AF.Identity, bias=bout_t[4][:, mj : mj + 1])
            nc.mark("evjx")
            drain(gen_projS(wout[4], HD, lambda kt, p: jacc[:, 0:2, kt, :],
                            ev_jx, npair=1))
            g1.drain()
            nc.gpsimd.tensor_tensor(out=mst[:], in0=gate_s[:], in1=mst[:],
                                    op=ALU.mult)
            nc.gpsimd.tensor_tensor(out=mtj[:], in0=gate_s[:], in1=mtj[:],
                                    op=ALU.mult)
            gate_j = T(gatep, [128, 8, N], BF16, "big")
            nc.mark("gate2")
            drain(gen_gate(2, mjt, mjs, gate_j))
            nc.vector.tensor_tensor(out=mjt[:], in0=gate_j[:], in1=mjt[:],
                                    op=ALU.mult)
            nc.vector.tensor_tensor(out=mjs[:], in0=gate_j[:], in1=mjs[:],
                                    op=ALU.mult)
            fs = [mts, mst, f2, mtj, mjt, mjs]

            # --- tail: wo1/wo2/final LN, next pass's head pumped as filler
            nxt = Feed()
            if c + 1 < NP:
                nxt.add(head_gens[c + 1])
            nc.mark("tail_wo1")
            nxt.pump(3)
            h1 = T(expp, [128, 8, 4, N], BF16, "exp")
            def ev_h1(mj0, gm, ps):
                for mi in range(gm):
                    m = mj0 + mi
                    nc.scalar.activation(
                        h1[:, m % 8, m // 8, :], ps[mi][:, 0:256],
                        AF.Relu, bias=bo1_t[:, m : m + 1])
            g = gen_projM(wo1, 2 * HD, 6 * HD,
                          lambda kt: fs[kt // 8][:, kt % 8, :], ev_h1)
            i = 0
            for _ in g:
                i += 1
                if i % 4 == 0:
                    nxt.pump(1)
            nc.mark("wo2")
            h2 = T(accp, [128, 8, N], BF16, "big")
            nxt.pump(2)
            g = gen_projM(wo2, HD, 2 * HD,
                          lambda kt: h1[:, kt % 8, kt // 8, :],
                          ev_split(lambda mj: h2[:, mj, :], bo2_t,
                                   lambda mj: mj))
            for _ in g:
                nxt.pump(1)

            # final LN (g,b pre-scaled by res_w) + (1-res_w)/2*(xt+xs)
            nc.mark("final_ln")
            yt = T(qkp, [128, 8, N], BF16, "qk")
            ln_norm(h2[:], lng_t[2], lnb_t[2], lambda mt: yt[:, mt, :],
                    feed=nxt)
            nxt.pump(4)
            nc.vector.tensor_add(out=yt[:], in0=yt[:], in1=osum[:])
            nc.sync.dma_start(out=y_d[:, :, slice(c * N, (c + 1) * N)],
                              in_=yt[:])
            nxt.drain()

    nc.compile()
    return nc


def _sel_const():
    s = np.zeros((8, 8 * 128), np.float32)
    for mt in range(8):
        s[mt, mt * 128 : (mt + 1) * 128] = 1.0
    return s.astype(BF)


def _o32_const():
    o = np.zeros((128, 64), np.float32)
    o[:, 32] = 1.0
    return o.astype(BF)


def _wl(w):
    """torch-style [M_out, K_in] -> pair-blocked [128, M/256, K/128, 256] bf16
    (one 2-mtile all-kt block contiguous per partition)."""
    a = np.asarray(w, np.float32).T          # [K, M]
    K, M = a.shape
    a = a.reshape(K // 128, 128, M // 256, 256).transpose(1, 2, 0, 3)
    return np.ascontiguousarray(a).astype(BF)


def _prep_inputs(i):
    res_w = float(np.asarray(i["res_w"]).reshape(-1)[0])
    sc = 1.0 / math.sqrt(128.0)

    shared = {
        "wexp0": _wl(i["exp_t_w"]), "wexp1": _wl(i["exp_s_w"]),
        "bexp0": _bias_cols(np.asarray(i["exp_t_b"]) + np.asarray(i["pos_enc"]).reshape(-1)),
        "bexp1": _bias_cols(np.asarray(i["exp_s_b"]) + np.asarray(i["pos_enc"]).reshape(-1)),
        "wjoint": _wl(np.asarray(i["joint_w"], np.float32) * 0.25),
        "bjoint": _bias_cols(i["joint_b"]),
        "wo1": _wl(i["out1_w"]), "bo1": _bias_cols(i["out1_b"]),
        "wo2": _wl(i["out2_w"]), "bo2": _bias_cols(i["out2_b"]),
        "sel_c": _sel_const(), "o32_c": _o32_const(),
    }
    for g in range(3):
        shared[f"wgate{g}"] = _wl(i["gate_w"][g])
        shared[f"bgate{g}"] = _bias_cols(i["gate_b"][g])
    for m in range(5):
        w = np.asarray(i["mha_in_w"][m], np.float32).copy()
        b = np.asarray(i["mha_in_b"][m], np.float32).copy()
        w[:HD] *= sc
        b[:HD] *= sc
        shared[f"wqkv{m}"] = _wl(w)
        shared[f"bqkv{m}"] = _bias_cols(b)
        wo = np.asarray(i["mha_out_w"][m], np.float32)
        if m in (2, 3):
            wo = wo * 0.25      # fold mean over the 4 query positions
        shared[f"wout{m}"] = _wl(wo)
        shared[f"bout{m}"] = _bias_cols(i["mha_out_b"][m])
    for ln in range(3):
        g = np.asarray(i["ln_g"][ln], np.float32)
        b = np.asarray(i["ln_b"][ln], np.float32)
        if ln == 2:
            g = g * res_w
            b = b * res_w
        shared[f"lng{ln}"] = _bias_cols(g)
        shared[f"lnb{ln}"] = _bias_cols(b)

    def shard_x(x, c):
        xc = np.asarray(x, np.float32)[c * BC : (c + 1) * BC, 0, :]  # [512,1024]
        return np.ascontiguousarray(xc.T.reshape(8, 128, BC).transpose(1, 0, 2))

    in_maps = []
    for c in range(NCORES):
        m = dict(shared)
        m["xt"] = shard_x(i["temporal_features"], c)
        m["xs"] = shard_x(i["spatial_features"], c)
        in_maps.append(m)
    return res_w, in_maps


def kernel(**inputs):
    res_w, in_maps = _prep_inputs(inputs)
    nc = build(res_w)
    res = bass_utils.run_bass_kernel_spmd(nc, in_maps, core_ids=list(range(NCORES)))
    outs = []
    for c in range(NCORES):
        y = res.results[c]["y"]                                   # [128,8,512]
        outs.append(np.asarray(y).astype(np.float32)
                    .transpose(1, 0, 2).reshape(HD, BC).T)
    return np.concatenate(outs, 0)[:, None, :].astype(np.float32)


# revision 26
# speedup vs baseline: 1.0649x; 1.0086x over previous
"""Trainium2 Bass kernel for nn_MISA (dense_transformer, data-parallel over 8 cores).

Layout: feature-major activations [feat_part=128, mtile, batch_cols] per core.
Batch 4096 -> 512 per core -> two passes of 256 columns.
All matmuls bf16 (fp32 PSUM accumulation); LN/softmax internals fp32.

v3 (PE-gap removal): the PE is the bottleneck engine; v2 lost ~400us to PE
idle gaps waiting on DVE softmax/LN chains plus ~300us of HAM cold-throttle
restarts those gaps caused.  v3 keeps the PE warm:
- every projection is a generator yielding at m-tile boundaries; a Feed
  object interleaves pending projection chunks into every DVE-bound window
  (softmax, LayerNorm, gate chains) as PE filler.
- attention loops are software-pipelined: scores(e1) / softmax(e1) /
  filler / av(e1-1), so av's dependency on softmax is always satisfied by
  the time the PE reaches it.
- av accumulation adds run on GpSimd (otherwise idle), LN's per-tile
  affine (g,b) runs on the Scalar engine, self-attn residual adds moved
  from PE identity-matmuls to DVE.
- pass c+1's head (x load, expand, kv0, q0) is pumped as PE filler during
  pass c's tail (wo1/wo2/final LN), removing the inter-pass PE gap.

Structural simplifications (exact, not approximations):
- attention with all-equal keys/values (q/k/v = broadcast joint row) is the
  identity on v: cross_tj == cross_sj == out_proj4(v_proj4(joint)).
- mean over query positions commutes with out_proj and with A@V, so the six
  cross outputs never materialize per-query outputs (abar-weighted V only).
- all-equal queries (j as q): single query row, output equals its mean.
"""
import sys, math
from contextlib import ExitStack
sys.path.insert(0, "/opt/trn_rl_repo")

import numpy as np
import ml_dtypes

import concourse.bass as bass
import concourse.mybir as mybir
from concourse import bacc
import concourse.tile as tile
from concourse import bass_utils

F32 = mybir.dt.float32
BF16 = mybir.dt.bfloat16
FP8E3 = mybir.dt.float8e3
AF = mybir.ActivationFunctionType
ALU = mybir.AluOpType
BF = ml_dtypes.bfloat16

H = 8
E = 4
HD = 1024
B = 4096
NCORES = 8
BC = B // NCORES          # 512 batch per core
NP = 2                    # passes per core
N = BC // NP              # 256 batch cols per pass
EPS = 1e-5


def _bias_cols(b):
    # [M] -> [128, M//128]: column m = per-partition bias of m-tile m
    return np.ascontiguousarray(np.asarray(b, np.float32).reshape(-1, 128).T)


class Feed:
    """Queue of projection generators (or factories) pumped as PE filler.
    Factories instantiate lazily when reached, so their eager first weight
    DMA overlaps the tail of the preceding stream."""
    def __init__(self, *gens):
        self.q = list(gens)

    def add(self, g):
        self.q.append(g)

    def pump(self, n=1):
        while n > 0 and self.q:
            g = self.q[0]
            if not hasattr(g, "__next__"):
                g = self.q[0] = g()
            try:
                next(g)
                n -= 1
            except StopIteration:
                self.q.pop(0)

    def drain(self):
        while self.q:
            self.pump(1)


def drain(g):
    for _ in g:
        pass


def build(res_w: float):
    nc = bacc.Bacc("TRN2", target_bir_lowering=False, debug=False)

    def din(name, shape, dt):
        return nc.dram_tensor(name, list(shape), dt, kind="ExternalInput").ap()

    xt_d = din("xt", (128, 8, BC), F32)
    xs_d = din("xs", (128, 8, BC), F32)
    # weights pair-blocked: [128, M/256, K/128, 256] — one (2-mtile, all-kt)
    # block is contiguous per partition, so block DMAs run at full rate
    wexp = [din(f"wexp{j}", (128, 16, 8, 256), BF16) for j in range(2)]
    bexp = [din(f"bexp{j}", (128, 32), F32) for j in range(2)]
    wqkv = [din(f"wqkv{i}", (128, 12, 8, 256), BF16) for i in range(5)]
    bqkv = [din(f"bqkv{i}", (128, 24), F32) for i in range(5)]
    wout = [din(f"wout{i}", (128, 4, 8, 256), BF16) for i in range(5)]
    bout = [din(f"bout{i}", (128, 8), F32) for i in range(5)]
    wjoint = din("wjoint", (128, 4, 16, 256), BF16)
    bjoint = din("bjoint", (128, 8), F32)
    wgate = [din(f"wgate{g}", (128, 4, 16, 256), BF16) for g in range(3)]
    bgate = [din(f"bgate{g}", (128, 8), F32) for g in range(3)]
    wo1 = din("wo1", (128, 8, 48, 256), FP8E3)
    bo1 = din("bo1", (128, 16), F32)
    wo2 = din("wo2", (128, 4, 16, 256), BF16)
    bo2 = din("bo2", (128, 8), F32)
    lng = [din(f"lng{i}", (128, 8), F32) for i in range(3)]
    lnb = [din(f"lnb{i}", (128, 8), F32) for i in range(3)]
    sel_d = din("sel_c", (8, 8 * 128), BF16)
    o32_d = din("o32_c", (128, 64), BF16)
    y_d = nc.dram_tensor("y", [128, 8, BC], BF16, kind="ExternalOutput").ap()

    with tile.TileContext(nc) as tc, ExitStack() as ctx:
        P = lambda **kw: ctx.enter_context(tc.tile_pool(**kw))
        cst = P(name="cst", bufs=1)
        wgp = P(name="wgp", bufs=3)                 # [128,8,256] weight blocks
        mmp = P(name="mmp", bufs=4, space="PSUM")   # 4 x [128,512] banks
        scp = P(name="scp", bufs=1, space="PSUM")   # [8,4,N] = 2 banks
        brp = P(name="brp", bufs=2, space="PSUM")   # 2 x [128,N] banks
        expp = P(name="expp", bufs=2)   # xp_t, xp_s; also h1, next xp_t
        enhp = P(name="enhp", bufs=2)   # t_enh, s_enh [128,8,4,N]
        qkvp = P(name="qkvp", bufs=2)   # k4, v4 [128,4,8,N]
        acc4p = P(name="acc4p", bufs=2)  # qa tiles + jacc [128,4,8,N]
        bigp = P(name="bigp", bufs=7)   # all long-lived [128,8,N] bf16
        actp = meanp = accp = gatep = bigp
        qkp = P(name="qkp", bufs=2)     # qk products, av curs, ln scratch
        xbp = P(name="xbp", bufs=3)     # xt_b, xs_b, osum
        smp = P(name="smp", bufs=1)     # softmax exp [8,4,N] bf16
        smdp = P(name="smdp", bufs=2)   # softmax denom [8,N] f32
        smbp = P(name="smbp", bufs=3)   # a_l bf16 [8,N]
        smrp = P(name="smrp", bufs=3)   # LN scalars [1,N] f32
        smabp = P(name="smabp", bufs=4)  # abar f32 [8,N]

        nc._phase_marks = []
        def mark(name):
            nc._phase_marks.append((name, nc.next_id()))
        nc.mark = mark

        _tc = [0]
        def T(pool, shape, dtype, tag):
            _tc[0] += 1
            return pool.tile(shape, dtype, tag=tag, name=f"{tag}_{_tc[0]}")

        ones_b = T(cst, [128, 1], BF16, "ones_b")
        nc.any.memset(ones_b[:], 1.0)
        onerow_f = T(cst, [1, 128], F32, "onerow_f")
        nc.any.memset(onerow_f[:], 1.0)
        sel = T(cst, [8, 8 * 128], BF16, "sel")
        nc.sync.dma_start(out=sel[:], in_=sel_d)
        o32 = T(cst, [128, 64], BF16, "o32")   # all-ones column at col 32
        nc.sync.dma_start(out=o32[:], in_=o32_d)
        eps_t = T(cst, [1, 1], F32, "eps_t")
        nc.any.memset(eps_t[:], EPS)

        def ctile(name, ap):
            t = cst.tile(list(ap.shape), ap.dtype, tag=name)
            nc.sync.dma_start(out=t[:], in_=ap)
            return t

        bexp_t = [ctile(f"bexp{j}", bexp[j]) for j in range(2)]
        bqkv_t = [ctile(f"bqkv{i}", bqkv[i]) for i in range(5)]
        bout_t = [ctile(f"bout{i}", bout[i]) for i in range(5)]
        bjoint_t = ctile("bjoint", bjoint)
        bgate_t = [ctile(f"bgate{g}", bgate[g]) for g in range(3)]
        bo1_t = ctile("bo1", bo1)
        bo2_t = ctile("bo2", bo2)
        lng_t = [ctile(f"lng{i}", lng[i]) for i in range(3)]
        lnb_t = [ctile(f"lnb{i}", lnb[i]) for i in range(3)]

        def gen_projS(w_d, M, src_pair, evict2, wcol0=0, npair=2):
            """Shared-weight projection, K=1024. src_pair(kt, p) -> [128,2,N]
            moving pair; two pairs (4 sources) per stationary load.
            evict2(mj, psums): psums[p] = [128,512] = pair p's two outputs.
            Yields once per mj (16 matmuls).  The first weight DMA issues
            eagerly at call time; later blocks prefetch one group ahead."""
            nmt = M // 128
            blocks = list(range(0, nmt, 2))
            wts = {}
            def load(mj0, split=False):
                wt = T(wgp, [128, 8, 256], BF16, "wg")
                blk = w_d[:, (wcol0 + mj0 * 128) // 256, :, :]
                if split:
                    nc.sync.dma_start(out=wt[:, 0:4, :], in_=blk[:, 0:4, :])
                    nc.sync.dma_start(out=wt[:, 4:8, :], in_=blk[:, 4:8, :])
                else:
                    nc.sync.dma_start(out=wt[:], in_=blk)
                wts[mj0] = wt
            load(blocks[0], split=True)
            def g():
                for bi, mj0 in enumerate(blocks):
                    gm = min(2, nmt - mj0)
                    wt = wts.pop(mj0)
                    for mj in range(mj0, mj0 + gm):
                        ps = [T(mmp, [128, 512], F32, "mm")
                              for _ in range(npair)]
                        for kt in range(8):
                            w_sl = wt[:, kt,
                                      (mj - mj0) * 128 : (mj - mj0 + 1) * 128]
                            for p in range(npair):
                                nc.tensor.matmul(ps[p][:], w_sl,
                                                 src_pair(kt, p),
                                                 start=(kt == 0),
                                                 stop=(kt == 7))
                        evict2(mj, ps)
                        if mj == mj0 and bi + 1 < len(blocks):
                            load(blocks[bi + 1])
                        yield
            return g()

        def gen_projM(w_d, M, K, src, evict2m, wcol0=0, fp8_scale=None):
            """Single-source projection. One PSUM bank per m-tile (interleaved
            accumulation groups must not share a bank: start=True clears the
            has-written bits bank-wide). evict2m(mj0, gm, ps_list).
            Yields once per 8-kt chunk (16 matmuls at gm=2).  First weight DMA
            issues eagerly at call time; later chunks prefetch one ahead.
            fp8_scale: weights stored e3m4 scaled by fp8_scale; staged through
            a qkp slot and upcast to bf16 on DVE (halves weight DMA bytes)."""
            nmt, nkt = M // 128, K // 128
            steps = [(mj0, kc0) for mj0 in range(0, nmt, 2)
                     for kc0 in range(0, nkt, 8)]
            wts = {}
            def load(i, split=False):
                mj0, kc0 = steps[i]
                kc = min(8, nkt - kc0)
                wt = T(wgp, [128, 8, 256], BF16, "wg")
                blk = w_d[:, (wcol0 + mj0 * 128) // 256, kc0 : kc0 + kc, :]
                if fp8_scale is not None:
                    w8 = T(qkp, [128, 8, 256], FP8E3, "qk")
                    nc.sync.dma_start(out=w8[:, :kc, :], in_=blk)
                    nc.vector.tensor_scalar_mul(wt[:, :kc, :], w8[:, :kc, :],
                                                1.0 / fp8_scale)
                elif split and kc == 8:
                    nc.sync.dma_start(out=wt[:, 0:4, :], in_=blk[:, 0:4, :])
                    nc.sync.dma_start(out=wt[:, 4:8, :], in_=blk[:, 4:8, :])
                else:
                    nc.sync.dma_start(out=wt[:, :kc, :], in_=blk)
                wts[i] = wt
            load(0, split=True)
            def g():
                ps = None
                for i, (mj0, kc0) in enumerate(steps):
                    gm = min(2, nmt - mj0)
                    if kc0 == 0:
                        ps = [T(mmp, [128, 512], F32, "mm") for _ in range(gm)]
                    kc = min(8, nkt - kc0)
                    wt = wts.pop(i)
                    if i + 1 < len(steps):
                        load(i + 1)
                    for kt in range(kc0, kc0 + kc):
                        s = src(kt)
                        for mi in range(gm):
                            nc.tensor.matmul(
                                ps[mi][:, 0:256],
                                wt[:, kt - kc0, mi * 128 : (mi + 1) * 128], s,
                                start=(kt == 0), stop=(kt == nkt - 1))
                    if kc0 + 8 >= nkt:
                        evict2m(mj0, gm, ps)
                    yield
            return g()

        def ev_split(dsts_of_mj, btile, bcol_of_mj, func=AF.Identity):
            """projM eviction: per-mtile ACT evicts [128,256] with bias."""
            def _ev(mj0, gm, ps):
                for mi in range(gm):
                    nc.scalar.activation(
                        dsts_of_mj(mj0 + mi), ps[mi][:, 0:256],
                        func, bias=btile[:, bcol_of_mj(mj0 + mi)
                                         : bcol_of_mj(mj0 + mi) + 1])
            return _ev

        def scores_all(q_sl, k4):
            """psum [8,4,N]: row h of col-block e2 = q[h].k[e2,h] (q pre-scaled).
            q_sl [128,8,N] contiguous; k4 [128,4,8,N] e-major."""
            sp = T(scp, [8, 4, N], F32, "sc")
            for e2 in range(4):
                p = T(qkp, [128, 8, N], BF16, "qk")
                nc.vector.tensor_tensor(
                    out=p[:], in0=q_sl, in1=k4[:, e2, :, :], op=ALU.mult)
                for kt in range(8):
                    nc.tensor.matmul(sp[:, e2, :], o32[:, 32 - kt : 40 - kt],
                                     p[:, kt, :], start=(kt == 0), stop=(kt == 7))
            return sp

        def softmax_tiles(sp):
            """sp [8,4,N] psum scores -> 4 bf16 [8,N] attention-weight tiles."""
            et = T(smp, [8, 4, N], BF16, "sm")
            nc.scalar.activation(et[:], sp[:], AF.Exp)
            d = T(smdp, [8, N], F32, "smd")
            nc.vector.tensor_add(out=d[:], in0=et[:, 0, :], in1=et[:, 1, :])
            for e2 in (2, 3):
                nc.vector.tensor_add(out=d[:], in0=d[:], in1=et[:, e2, :])
            r = T(smdp, [8, N], F32, "smd")
            nc.vector.reciprocal_approx_fast(out=r[:], in_=d[:])
            outs = []
            for e2 in range(4):
                a = T(smbp, [8, N], BF16, "smb")
                nc.vector.tensor_tensor(out=a[:], in0=et[:, e2, :], in1=r[:],
                                        op=ALU.mult)
                outs.append(a)
            return outs

        def av_accum(a_list, v4, dst_sl):
            """dst_sl [128,8,N] contiguous = sum_e2 bcast(a_list[e2]) * V[e2].
            v4 [128,4,8,N] e-major.  All DVE; one scratch cur reused so the
            adds interleave with the next e2's mults."""
            for e2 in range(4):
                cur = dst_sl if e2 == 0 else T(qkp, [128, 8, N], BF16, "qk")
                for mt in range(0, 8, 2):
                    bp = T(brp, [128, 2, N], F32, "br")
                    for q in range(2):
                        nc.tensor.matmul(
                            bp[:, q, :],
                            sel[:, (mt + q) * 128 : (mt + q + 1) * 128],
                            a_list[e2][:], start=True, stop=True)
                    nc.vector.tensor_tensor(
                        out=cur[:, mt : mt + 2, :], in0=bp[:],
                        in1=v4[:, e2, mt : mt + 2, :], op=ALU.mult)
                if e2 > 0:
                    nc.vector.tensor_add(out=dst_sl, in0=dst_sl, in1=cur[:])

        def ln_norm(x_sl, g_t, b_t, dst_of_mt, feed=None):
            """LayerNorm over the 1024 feats of x_sl [128,8,N] (bf16, in-place
            scratch); writes normalized*g+b to dst_of_mt(mt).  Per-mt chain:
            2 DVE ops (all-bf16 SBUF, 2x mode) + 1 Scalar affine."""
            sq = T(qkp, [128, 8, N], BF16, "qk")
            nc.vector.tensor_tensor(out=sq[:], in0=x_sl, in1=x_sl, op=ALU.mult)
            st_s = T(brp, [1, N], F32, "br")
            for kt in range(8):
                nc.tensor.matmul(st_s[:], ones_b[:], x_sl[:, kt, :],
                                 start=(kt == 0), stop=(kt == 7))
            st_q = T(brp, [1, N], F32, "br")
            for kt in range(8):
                nc.tensor.matmul(st_q[:], ones_b[:], sq[:, kt, :],
                                 start=(kt == 0), stop=(kt == 7))
            mean = T(smrp, [1, N], F32, "smr")
            nc.vector.tensor_scalar_mul(mean[:], st_s[:], 1.0 / HD)
            mb = T(brp, [128, N], F32, "br")
            nc.tensor.matmul(mb[:], onerow_f[:], mean[:], start=True, stop=True)
            msq = T(smrp, [1, N], F32, "smr")
            nc.vector.tensor_scalar_mul(msq[:], st_q[:], 1.0 / HD)
            var = T(smrp, [1, N], F32, "smr")
            nc.vector.tensor_tensor(out=var[:], in0=mean[:], in1=mean[:],
                                    op=ALU.mult)
            nc.vector.tensor_tensor(out=var[:], in0=msq[:], in1=var[:],
                                    op=ALU.subtract)
            std = T(smrp, [1, N], F32, "smr")
            nc.scalar.activation(std[:], var[:], AF.Sqrt, bias=eps_t[:])
            rstd = T(smrp, [1, N], F32, "smr")
            nc.vector.reciprocal_approx_fast(out=rstd[:], in_=std[:])
            rb = T(brp, [128, N], F32, "br")
            nc.tensor.matmul(rb[:], onerow_f[:], rstd[:], start=True, stop=True)
            # bf16 SBUF copies of the broadcasts
            mbb = T(qkp, [128, 2, N], BF16, "qk")
            nc.scalar.activation(mbb[:, 0, :], mb[:], AF.Identity, bias=0.0)
            nc.scalar.activation(mbb[:, 1, :], rb[:], AF.Identity, bias=0.0)
            if feed is not None:
                feed.pump(1)
            for mt in range(8):
                nc.vector.tensor_tensor(out=x_sl[:, mt, :], in0=x_sl[:, mt, :],
                                        in1=mbb[:, 0, :], op=ALU.subtract)
                nc.vector.tensor_tensor(out=x_sl[:, mt, :], in0=x_sl[:, mt, :],
                                        in1=mbb[:, 1, :], op=ALU.mult)
                nc.scalar.activation(
                    dst_of_mt(mt), x_sl[:, mt, :], AF.Identity,
                    bias=b_t[:, mt : mt + 1], scale=g_t[:, mt : mt + 1])

        def gen_kv(mi, src4, k4, v4):
            """K/V projection of mha mi from src4 [128,8(kt),4(e),N] ->
            k4, v4 [128,4(e),8(mt),N] e-major."""
            def ev(mj, ps):
                dst = k4 if mj < 8 else v4
                bcol = 8 + mj      # k tiles: cols 8..15, v tiles: 16..23
                for p in range(2):
                    for q in range(2):
                        nc.scalar.activation(
                            dst[:, 2 * p + q, mj % 8, :],
                            ps[p][:, q * 256 : (q + 1) * 256],
                            AF.Identity,
                            bias=bqkv_t[mi][:, bcol : bcol + 1])
            return gen_projS(
                wqkv[mi], 2 * HD,
                lambda kt, p: src4[:, kt, 2 * p : 2 * p + 2, :], ev, wcol0=HD)

        def gen_q(mi, src4, qa):
            """q projection into a merged q/acc tile: scores consume the e1
            slice, then av_accum overwrites it with the AV result in place."""
            def ev(mj, ps):
                for p in range(2):
                    for q in range(2):
                        nc.scalar.activation(
                            qa[:, 2 * p + q, mj, :],
                            ps[p][:, q * 256 : (q + 1) * 256],
                            AF.Identity, bias=bqkv_t[mi][:, mj : mj + 1])
            return gen_projS(
                wqkv[mi], HD,
                lambda kt, p: src4[:, kt, 2 * p : 2 * p + 2, :], ev)

        def gen_expand(j, x_b, xp):
            # expand: m-tile m = e*8+mj -> xp[:, mj, e, :]
            def ev_exp(mj0, gm, ps):
                for mi in range(gm):
                    m = mj0 + mi
                    nc.scalar.activation(
                        xp[:, m % 8, m // 8, :], ps[mi][:, 0:256],
                        AF.Identity, bias=bexp_t[j][:, m : m + 1])
            return gen_projM(wexp[j], E * HD, HD,
                             lambda kt: x_b[:, kt, :], ev_exp)

        def gen_self_out(j, qa, xp, enh_dst):
            """out proj -> enh_dst pre-LN; residual added on DVE."""
            def ev_out(mj, ps):
                for p in range(2):
                    nc.scalar.activation(
                        enh_dst[:, mj, 2 * p : 2 * p + 2, :], ps[p][:],
                        AF.Identity, bias=bout_t[j][:, mj : mj + 1])
                nc.vector.tensor_tensor(
                    out=enh_dst[:, mj, :, :], in0=enh_dst[:, mj, :, :],
                    in1=xp[:, mj, :, :], op=ALU.add)
            return gen_projS(
                wout[j], HD,
                lambda kt, p: qa[:, 2 * p : 2 * p + 2, kt, :], ev_out)

        def attn_loop(qa, k4, v4, feed, fill=2):
            """self-attn e1 loop, software-pipelined; av result replaces q in
            qa in place one iteration late."""
            prev = None
            for e1 in range(4):
                sp = scores_all(qa[:, e1, :, :], k4)
                a_l = softmax_tiles(sp)
                feed.pump(fill)
                if prev is not None:
                    av_accum(prev, v4, qa[:, e1 - 1, :, :])
                prev = a_l
            feed.pump(fill)
            av_accum(prev, v4, qa[:, 3, :, :])

        def cross_loop(qa, k4, feed, fill=2):
            """cross-attn e1 loop accumulating abar (mean attn weights)."""
            abar = [None] * 4
            for e1 in range(4):
                sp = scores_all(qa[:, e1, :, :], k4)
                a_l = softmax_tiles(sp)
                feed.pump(fill)
                for e2 in range(4):
                    if e1 == 0:
                        ab = T(smabp, [8, N], BF16, "smab")
                        nc.vector.tensor_copy(out=ab[:], in_=a_l[e2][:])
                        abar[e2] = ab
                    else:
                        nc.vector.tensor_add(out=abar[e2][:], in0=abar[e2][:],
                                             in1=a_l[e2][:])
            return abar

        def cross_fin(mi, abar, v4, dst, feed=None):
            """abar-weighted AV + out proj (wout pre-scaled 0.25)."""
            cacc = T(accp, [128, 8, N], BF16, "big")
            av_accum(abar, v4, cacc[:])
            if feed is not None:
                feed.pump(6)
            drain(gen_projM(wout[mi], HD, HD, lambda kt: cacc[:, kt, :],
                            ev_split(lambda mj: dst[:, mj, :], bout_t[mi],
                                     lambda mj: mj)))

        def gen_gate(g, in_a, in_b, gt):
            return gen_projM(
                wgate[g], HD, 2 * HD,
                lambda kt: in_a[:, kt, :] if kt < 8 else in_b[:, kt - 8, :],
                ev_split(lambda mj: gt[:, mj, :], bgate_t[g],
                         lambda mj: mj, func=AF.Sigmoid))

        def run_selfB(j, enh_dst, sum_dst, feed):
            """LN each position of enh_dst in place; sum_dst = sum_e enh."""
            for e1 in range(4):
                ln_norm(enh_dst[:, :, e1, :], lng_t[j], lnb_t[j],
                        lambda mt, e1=e1: enh_dst[:, mt, e1, :], feed=feed)
                feed.pump(2)
            t2 = T(qkp, [128, 8, N], BF16, "qk")
            nc.vector.tensor_add(out=sum_dst[:], in0=enh_dst[:, :, 0, :],
                                 in1=enh_dst[:, :, 1, :])
            nc.vector.tensor_add(out=t2[:], in0=enh_dst[:, :, 2, :],
                                 in1=enh_dst[:, :, 3, :])
            nc.vector.tensor_add(out=sum_dst[:], in0=sum_dst[:], in1=t2[:])

        def gen_head(c, out):
            """Pass head: x load + expand(0) + kv0 + q0.  Run via a Feed so
            pass c's head can fill pass c-1's tail."""
            bs = slice(c * N, (c + 1) * N)
            xt_b = T(xbp, [128, 8, N], BF16, "xb")
            xs_b = T(xbp, [128, 8, N], BF16, "xb")
            for xd, xb in ((xt_d, xt_b), (xs_d, xs_b)):
                for h in range(4):
                    xf = T(qkp, [128, 2, N], F32, "qk")
                    nc.sync.dma_start(out=xf[:],
                                      in_=xd[:, 2 * h : 2 * h + 2, bs])
                    nc.vector.tensor_copy(out=xb[:, 2 * h : 2 * h + 2, :],
                                          in_=xf[:])
                yield
            xp_t = T(expp, [128, 8, 4, N], BF16, "exp")
            yield from gen_expand(0, xt_b, xp_t)
            k4t = T(qkvp, [128, 4, 8, N], BF16, "qkv")
            v4t = T(qkvp, [128, 4, 8, N], BF16, "qkv")
            yield from gen_kv(0, xp_t, k4t, v4t)
            qa_t = T(acc4p, [128, 4, 8, N], BF16, "acc4")
            yield from gen_q(0, xp_t, qa_t)
            out.update(xt_b=xt_b, xs_b=xs_b, xp_t=xp_t, k4t=k4t, v4t=v4t,
                       qa_t=qa_t)

        heads = [{} for _ in range(NP)]
        head_gens = [gen_head(c, heads[c]) for c in range(NP)]
        drain(head_gens[0])

        for c in range(NP):
            hd = heads[c]
            xt_b, xs_b, xp_t = hd["xt_b"], hd["xs_b"], hd["xp_t"]
            k4t, v4t, qa_t = hd["k4t"], hd["v4t"], hd["qa_t"]

            nc.mark("mid_start")
            t_enh = T(enhp, [128, 8, 4, N], BF16, "enh")
            s_enh = T(enhp, [128, 8, 4, N], BF16, "enh")
            sum_t = T(actp, [128, 8, N], BF16, "big")
            sum_s = T(actp, [128, 8, N], BF16, "big")

            # --- self-t, expand(1) fills the softmax windows
            xp_s = T(expp, [128, 8, 4, N], BF16, "exp")
            f = Feed(gen_expand(1, xs_b, xp_s))
            nc.mark("attn_t")
            attn_loop(qa_t, k4t, v4t, f, fill=3)
            k4s = T(qkvp, [128, 4, 8, N], BF16, "qkv")
            v4s = T(qkvp, [128, 4, 8, N], BF16, "qkv")
            gkv1 = gen_kv(1, xp_s, k4s, v4s)
            f.drain()
            # residual precompute frees xt_b/xs_b before the tail
            cres = (1.0 - res_w) * 0.5
            osum = T(xbp, [128, 8, N], BF16, "xb")
            nc.vector.tensor_add(out=osum[:], in0=xt_b[:], in1=xs_b[:])
            nc.vector.tensor_scalar_mul(osum[:], osum[:], cres)
            # --- kv1 stream, then self-out t
            nc.mark("kv1")
            drain(gkv1)
            nc.mark("self_out0")
            drain(gen_self_out(0, qa_t, xp_t, t_enh))
            # --- q1 with t-LN interleaved
            qa_s = T(acc4p, [128, 4, 8, N], BF16, "acc4")
            f = Feed(gen_q(1, xp_s, qa_s))
            nc.mark("q1_selfB0")
            run_selfB(0, t_enh, sum_t, f)
            f.drain()
            # --- self-s, q2 fills the windows
            qa_c2 = T(acc4p, [128, 4, 8, N], BF16, "acc4")
            f = Feed(gen_q(2, t_enh, qa_c2))
            nc.mark("attn_s")
            attn_loop(qa_s, k4s, v4s, f, fill=2)
            f.drain()
            # --- kv3 chunks cover the last av's adds before self-out s
            k4c3 = T(qkvp, [128, 4, 8, N], BF16, "qkv")
            v4c3 = T(qkvp, [128, 4, 8, N], BF16, "qkv")
            f = Feed(gen_kv(3, t_enh, k4c3, v4c3))
            f.pump(2)
            nc.mark("self_out1")
            drain(gen_self_out(1, qa_s, xp_s, s_enh))
            nc.mark("kv3_selfB1")
            run_selfB(1, s_enh, sum_s, f)
            f.drain()
            # --- q3 stream
            qa_c3 = T(acc4p, [128, 4, 8, N], BF16, "acc4")
            nc.mark("q3")
            drain(gen_q(3, s_enh, qa_c3))
            # --- cross st (mha3): joint/vj/qj fill the windows
            joint = T(actp, [128, 8, N], BF16, "big")
            vj = T(actp, [128, 8, N], BF16, "big")
            qj = T(actp, [128, 8, N], BF16, "big")
            mst = T(meanp, [128, 8, N], BF16, "big")
            mts = T(meanp, [128, 8, N], BF16, "big")
            f = Feed(
                gen_projM(wjoint, HD, 2 * HD,
                          lambda kt: sum_t[:, kt, :] if kt < 8
                          else sum_s[:, kt - 8, :],
                          ev_split(lambda mj: joint[:, mj, :], bjoint_t,
                                   lambda mj: mj)),
                lambda: gen_projM(wqkv[4], HD, HD, lambda kt: joint[:, kt, :],
                                  ev_split(lambda mj: vj[:, mj, :], bqkv_t[4],
                                           lambda mj: 16 + mj),
                                  wcol0=2 * HD),
                lambda: gen_projM(wqkv[4], HD, HD, lambda kt: joint[:, kt, :],
                                  ev_split(lambda mj: qj[:, mj, :], bqkv_t[4],
                                           lambda mj: mj)))
            nc.mark("cross_loop3")
            abar3 = cross_loop(qa_c3, k4c3, f, fill=3)
            f.drain()
            # --- finish cross st; kv2 chunks fill the av window
            k4c2 = T(qkvp, [128, 4, 8, N], BF16, "qkv")
            v4c2 = T(qkvp, [128, 4, 8, N], BF16, "qkv")
            fkv2 = Feed(gen_kv(2, s_enh, k4c2, v4c2))
            nc.mark("crossfin3_kv2")
            cross_fin(3, abar3, v4c3, mst, feed=fkv2)
            fkv2.drain()
            # --- cross ts (mha2): mtj fills the windows
            mtj = T(meanp, [128, 8, N], BF16, "big")
            f = Feed(gen_projM(wout[4], HD, HD, lambda kt: vj[:, kt, :],
                               ev_split(lambda mj: mtj[:, mj, :], bout_t[4],
                                        lambda mj: mj)))
            nc.mark("cross_loop2_mtj")
            abar2 = cross_loop(qa_c2, k4c2, f, fill=1)
            f.drain()
            # --- finish cross ts; kv4t chunks fill the av window
            k4j1 = T(qkvp, [128, 4, 8, N], BF16, "qkv")
            v4j1 = T(qkvp, [128, 4, 8, N], BF16, "qkv")
            fkv4 = Feed(gen_kv(4, t_enh, k4j1, v4j1))
            nc.mark("crossfin2_kv4t")
            cross_fin(2, abar2, v4c2, mts, feed=fkv4)
            fkv4.drain()
            gate_t = T(gatep, [128, 8, N], BF16, "big")
            g0 = Feed(gen_gate(0, mts, mtj, gate_t))
            # --- jx: single-query cross-attn (q = joint row)
            jacc = T(acc4p, [128, 4, 8, N], BF16, "acc4")
            k4j2 = T(qkvp, [128, 4, 8, N], BF16, "qkv")
            v4j2 = T(qkvp, [128, 4, 8, N], BF16, "qkv")
            fkv = Feed(gen_kv(4, s_enh, k4j2, v4j2))
            nc.mark("jx1")
            a_l1 = softmax_tiles(scores_all(qj[:], k4j1))
            fkv.pump(4)
            g0.pump(2)
            av_accum(a_l1, v4j1, jacc[:, 0, :, :])
            fkv.drain()
            nc.mark("jx2")
            a_l2 = softmax_tiles(scores_all(qj[:], k4j2))
            g0.drain()
            # gate_t consumed immediately so gate_s can reuse the pool slot
            f2 = T(accp, [128, 8, N], BF16, "big")
            nc.gpsimd.tensor_tensor(out=f2[:], in0=gate_t[:], in1=mtj[:],
                                    op=ALU.mult)
            nc.gpsimd.tensor_tensor(out=mts[:], in0=gate_t[:], in1=mts[:],
                                    op=ALU.mult)
            gate_s = T(gatep, [128, 8, N], BF16, "big")
            g1 = Feed(gen_gate(1, mst, mtj, gate_s))
            g1.pump(3)
            av_accum(a_l2, v4j2, jacc[:, 1, :, :])
            g1.pump(3)
            mjt = T(meanp, [128, 8, N], BF16, "big")
            mjs = T(meanp, [128, 8, N], BF16, "big")
            def ev_jx(mj, ps):
                for jj, dst in enumerate((mjt, mjs)):
                    nc.scalar.activation(
                        dst[:, mj, :], ps[0][:, jj * 256 : (jj + 1) * 256],
                        AF.Identity, bias=bout_t[4][:, mj : mj + 1])
            nc.mark("evjx")
            drain(gen_projS(wout[4], HD, lambda kt, p: jacc[:, 0:2, kt, :],
                            ev_jx, npair=1))
            g1.drain()
            nc.gpsimd.tensor_tensor(out=mst[:], in0=gate_s[:], in1=mst[:],
                                    op=ALU.mult)
            nc.gpsimd.tensor_tensor(out=mtj[:], in0=gate_s[:], in1=mtj[:],
                                    op=ALU.mult)
            gate_j = T(gatep, [128, 8, N], BF16, "big")
            nc.mark("gate2")
            drain(gen_gate(2, mjt, mjs, gate_j))
            nc.vector.tensor_tensor(out=mjt[:], in0=gate_j[:], in1=mjt[:],
                                    op=ALU.mult)
            nc.vector.tensor_tensor(out=mjs[:], in0=gate_j[:], in1=mjs[:],
                                    op=ALU.mult)
            fs = [mts, mst, f2, mtj, mjt, mjs]

            # --- tail: wo1/wo2/final LN, next pass's head pumped as filler
            nxt = Feed()
            if c + 1 < NP:
                nxt.add(head_gens[c + 1])
            nc.mark("tail_wo1")
            nxt.pump(3)
            h1 = T(expp, [128, 8, 4, N], BF16, "exp")
            def ev_h1(mj0, gm, ps):
                for mi in range(gm):
                    m = mj0 + mi
                    nc.scalar.activation(
                        h1[:, m % 8, m // 8, :], ps[mi][:, 0:256],
                        AF.Relu, bias=bo1_t[:, m : m + 1])
            g = gen_projM(wo1, 2 * HD, 6 * HD,
                          lambda kt: fs[kt // 8][:, kt % 8, :], ev_h1,
                          fp8_scale=128.0)
            i = 0
            for _ in g:
                i += 1
                if i % 4 == 0:
                    nxt.pump(1)
            nc.mark("wo2")
            h2 = T(accp, [128, 8, N], BF16, "big")
            nxt.pump(2)
            g = gen_projM(wo2, HD, 2 * HD,
                          lambda kt: h1[:, kt % 8, kt // 8, :],
                          ev_split(lambda mj: h2[:, mj, :], bo2_t,
                                   lambda mj: mj))
            for _ in g:
                nxt.pump(1)

            # final LN (g,b pre-scaled by res_w) + (1-res_w)/2*(xt+xs)
            nc.mark("final_ln")
            yt = T(qkp, [128, 8, N], BF16, "qk")
            ln_norm(h2[:], lng_t[2], lnb_t[2], lambda mt: yt[:, mt, :],
                    feed=nxt)
            nxt.pump(4)
            nc.vector.tensor_add(out=yt[:], in0=yt[:], in1=osum[:])
            nc.sync.dma_start(out=y_d[:, :, slice(c * N, (c + 1) * N)],
                              in_=yt[:])
            nxt.drain()

    nc.compile()
    return nc


def _sel_const():
    s = np.zeros((8, 8 * 128), np.float32)
    for mt in range(8):
        s[mt, mt * 128 : (mt + 1) * 128] = 1.0
    return s.astype(BF)


def _o32_const():
    o = np.zeros((128, 64), np.float32)
    o[:, 32] = 1.0
    return o.astype(BF)


def _wl8(w, scale):
    """_wl but quantized to fp8 e3m4 with the given scale."""
    a = np.asarray(w, np.float32).T * scale
    K, M = a.shape
    a = a.reshape(K // 128, 128, M // 256, 256).transpose(1, 2, 0, 3)
    a = np.clip(np.ascontiguousarray(a), -15.5, 15.5)
    return a.astype(ml_dtypes.float8_e3m4)


def _wl(w):
    """torch-style [M_out, K_in] -> pair-blocked [128, M/256, K/128, 256] bf16
    (one 2-mtile all-kt block contiguous per partition)."""
    a = np.asarray(w, np.float32).T          # [K, M]
    K, M = a.shape
    a = a.reshape(K // 128, 128, M // 256, 256).transpose(1, 2, 0, 3)
    return np.ascontiguousarray(a).astype(BF)


def _prep_inputs(i):
    res_w = float(np.asarray(i["res_w"]).reshape(-1)[0])
    sc = 1.0 / math.sqrt(128.0)

    shared = {
        "wexp0": _wl(i["exp_t_w"]), "wexp1": _wl(i["exp_s_w"]),
        "bexp0": _bias_cols(np.asarray(i["exp_t_b"]) + np.asarray(i["pos_enc"]).reshape(-1)),
        "bexp1": _bias_cols(np.asarray(i["exp_s_b"]) + np.asarray(i["pos_enc"]).reshape(-1)),
        "wjoint": _wl(np.asarray(i["joint_w"], np.float32) * 0.25),
        "bjoint": _bias_cols(i["joint_b"]),
        "wo1": _wl8(i["out1_w"], 128.0), "bo1": _bias_cols(i["out1_b"]),
        "wo2": _wl(i["out2_w"]), "bo2": _bias_cols(i["out2_b"]),
        "sel_c": _sel_const(), "o32_c": _o32_const(),
    }
    for g in range(3):
        shared[f"wgate{g}"] = _wl(i["gate_w"][g])
        shared[f"bgate{g}"] = _bias_cols(i["gate_b"][g])
    for m in range(5):
        w = np.asarray(i["mha_in_w"][m], np.float32).copy()
        b = np.asarray(i["mha_in_b"][m], np.float32).copy()
        w[:HD] *= sc
        b[:HD] *= sc
        shared[f"wqkv{m}"] = _wl(w)
        shared[f"bqkv{m}"] = _bias_cols(b)
        wo = np.asarray(i["mha_out_w"][m], np.float32)
        if m in (2, 3):
            wo = wo * 0.25      # fold mean over the 4 query positions
        shared[f"wout{m}"] = _wl(wo)
        shared[f"bout{m}"] = _bias_cols(i["mha_out_b"][m])
    for ln in range(3):
        g = np.asarray(i["ln_g"][ln], np.float32)
        b = np.asarray(i["ln_b"][ln], np.float32)
        if ln == 2:
            g = g * res_w
            b = b * res_w
        shared[f"lng{ln}"] = _bias_cols(g)
        shared[f"lnb{ln}"] = _bias_cols(b)

    def shard_x(x, c):
        xc = np.asarray(x, np.float32)[c * BC : (c + 1) * BC, 0, :]  # [512,1024]
        return np.ascontiguousarray(xc.T.reshape(8, 128, BC).transpose(1, 0, 2))

    in_maps = []
    for c in range(NCORES):
        m = dict(shared)
        m["xt"] = shard_x(i["temporal_features"], c)
        m["xs"] = shard_x(i["spatial_features"], c)
        in_maps.append(m)
    return res_w, in_maps


def kernel(**inputs):
    res_w, in_maps = _prep_inputs(inputs)
    nc = build(res_w)
    res = bass_utils.run_bass_kernel_spmd(nc, in_maps, core_ids=list(range(NCORES)))
    outs = []
    for c in range(NCORES):
        y = res.results[c]["y"]                                   # [128,8,512]
        outs.append(np.asarray(y).astype(np.float32)
                    .transpose(1, 0, 2).reshape(HD, BC).T)
    return np.concatenate(outs, 0)[:, None, :].astype(np.float32)
